# revision 38
# baseline (speedup 1.0000x reference)
"""Trainium2 Bass kernel for nn_Encoder (6-layer causal transformer encoder).

Sharding: 8 cores = 4 batch elements x 2-core tensor-parallel pairs.
Within a pair: attention is head-split (4 of 8 heads per core), FFN/LN/residual
are token-split (1024 of 2048 tokens per core).  Rank asymmetry is expressed
purely through ReduceScatter / AllGather rank order, so the SPMD program is
identical on every core.

v2 changes vs baseline:
 - collectives split into 2 token-chunks and reordered so they overlap
   compute (AG chunk issued right after its LN-B chunk; RS chunk issued
   after the two q-groups that feed it, overlapping the other two).
 - LayerNorm + softmax normalization broadcast via PE rank-1 outer products
   instead of DRAM round-trips.
 - w1/w2 SBUF-resident per layer (single load, no streaming stalls).
 - LN statistics matmuls run on float32r data at full PE rate (no bf16 copy).
 - score matmuls restricted to causally-valid columns in the diagonal band.
 - HW is PE-bound at the ~1.2GHz p-state, so the schedule keeps matmul
   streams dense and avoids GPSIMD software ops (slower on HW than modeled).
"""

import os
import sys

sys.path.insert(0, "/opt/trn_rl_repo")

import numpy as np
import ml_dtypes

import concourse.bass as bass
import concourse.mybir as mybir
import concourse.tile as tile
from concourse import bacc, bass_utils
from concourse.masks import make_identity, make_upper_triangular

# Problem constants (hardcoded per harness contract).
B, S, V, D, F, L = 4, 2048, 32000, 512, 2048, 6
H, Dh = 8, 64
HL = H // 2            # local heads per core (4)
DL = HL * Dh           # 256 local head-dims
TOWN = S // 2          # 1024 tokens owned per core
P = 128
CC = D // P            # 4 c-chunks
FC = F // P            # 16 f-chunks
CH = 512               # token chunk for collectives / LN / FFN
NCH = TOWN // CH       # 2 chunks
LN_EPS = 1e-5

FP32 = mybir.dt.float32
FP32R = mybir.dt.float32r
BF16 = mybir.dt.bfloat16
I32 = mybir.dt.int32

GROUPS = [[0, 1], [2, 3], [4, 5], [6, 7]]

_CACHED = {}


def _build_program(no_cc=False):
    nc = bacc.Bacc("TRN2", target_bir_lowering=False, debug=False, num_devices=8)
    if no_cc:
        # benchmarking variant: collectives replaced by a local DRAM copy
        # (wrong results; identical compute/DMA structure)
        def fake_cc(kind, op, replica_groups, ins, outs, **kw):
            src = ins[0]
            dst = outs[0]
            n = min(src.size(), dst.size())
            nc.sync.dma_start(
                out=bass.AP(tensor=dst.tensor, offset=dst.offset, ap=[[1, n]]),
                in_=bass.AP(tensor=src.tensor, offset=src.offset, ap=[[1, n]]))

        nc.gpsimd.collective_compute = fake_cc

    D_ = {}
    D_["src"] = nc.dram_tensor("src", [TOWN], I32, kind="ExternalInput")
    D_["emb"] = nc.dram_tensor("emb", [V, D], FP32, kind="ExternalInput")
    D_["wq"] = nc.dram_tensor("wq", [L, D, DL], BF16, kind="ExternalInput")
    D_["wk"] = nc.dram_tensor("wk", [L, D, DL], BF16, kind="ExternalInput")
    D_["wv"] = nc.dram_tensor("wv", [L, D, DL], BF16, kind="ExternalInput")
    D_["wo"] = nc.dram_tensor("wo", [L, DL, D], BF16, kind="ExternalInput")
    D_["bq"] = nc.dram_tensor("bq", [L, DL], FP32, kind="ExternalInput")
    D_["bk"] = nc.dram_tensor("bk", [L, DL], FP32, kind="ExternalInput")
    D_["bv"] = nc.dram_tensor("bv", [L, DL], FP32, kind="ExternalInput")
    D_["bo"] = nc.dram_tensor("bo", [L, D], FP32, kind="ExternalInput")
    D_["w1"] = nc.dram_tensor("w1", [L, D, F], BF16, kind="ExternalInput")
    D_["b1"] = nc.dram_tensor("b1", [L, F], FP32, kind="ExternalInput")
    D_["w2"] = nc.dram_tensor("w2", [L, F, D], BF16, kind="ExternalInput")
    D_["b2"] = nc.dram_tensor("b2", [L, D], FP32, kind="ExternalInput")
    D_["ln_g"] = nc.dram_tensor("ln_g", [D], FP32, kind="ExternalInput")
    D_["ln_b"] = nc.dram_tensor("ln_b", [D], FP32, kind="ExternalInput")
    D_["out"] = nc.dram_tensor("out", [TOWN, D], FP32, kind="ExternalOutput")

    # DRAM scratch, chunk-major so each collective chunk is contiguous.
    D_["xh"] = [nc.dram_tensor(f"xh{l}", [NCH, D, CH], BF16, kind="Internal")
                for l in range(L)]
    D_["xf"] = [nc.dram_tensor(f"xf{l}", [NCH, 2, D, CH], BF16, kind="Internal")
                for l in range(L)]
    D_["apart"] = [nc.dram_tensor(f"apart{l}", [NCH, 2, D, CH], BF16,
                                  kind="Internal")
                   for l in range(L)]
    D_["aown"] = [nc.dram_tensor(f"aown{l}", [NCH, D, CH], BF16, kind="Internal")
                  for l in range(L)]

    with tile.TileContext(nc) as tc:
        _emit(nc, tc, D_)

    nc.compile()
    return nc


def _emit(nc, tc, D_):
    from contextlib import ExitStack

    no_pool = bool(int(os.environ.get("BASS_ENC_NOPOOL", "1")))
    fine_attn = bool(int(os.environ.get("BASS_ENC_FINEATTN", "0")))
    ln_row = bool(int(os.environ.get("BASS_ENC_LNROW", "0")))

    ctx = ExitStack()
    Exp = mybir.ActivationFunctionType.Exp
    Relu = mybir.ActivationFunctionType.Relu
    Copy = mybir.ActivationFunctionType.Copy
    Ident = mybir.ActivationFunctionType.Identity
    Sqrt = mybir.ActivationFunctionType.Sqrt
    ADD = mybir.AluOpType.add
    SUB = mybir.AluOpType.subtract

    consts = ctx.enter_context(tc.tile_pool(name="consts", bufs=1))
    wpool = ctx.enter_context(tc.tile_pool(name="weights", bufs=1))
    stream = ctx.enter_context(tc.tile_pool(name="stream", bufs=1))
    acts = ctx.enter_context(tc.tile_pool(name="acts", bufs=1))
    xstrm = ctx.enter_context(tc.tile_pool(name="xstrm", bufs=2))
    chk = ctx.enter_context(tc.tile_pool(name="chunks", bufs=2))
    small = ctx.enter_context(tc.tile_pool(name="small", bufs=1))
    scratch = ctx.enter_context(tc.tile_pool(name="scratch", bufs=2))
    h1pool = ctx.enter_context(tc.tile_pool(name="h1pool", bufs=1))
    expp = ctx.enter_context(tc.tile_pool(name="exp", bufs=4))
    psA = ctx.enter_context(tc.tile_pool(
        name="psA",
        bufs=4 if bool(int(os.environ.get("BASS_ENC_FINEATTN", "0"))) else 2,
        space="PSUM"))
    psB = ctx.enter_context(tc.tile_pool(name="psB", bufs=2, space="PSUM"))
    psC = ctx.enter_context(tc.tile_pool(name="psC", bufs=2, space="PSUM"))

    # ---- constants ----
    ident = consts.tile([P, P], FP32)
    make_identity(nc, ident)
    trimask = consts.tile([P, P], BF16)  # 1 where k<=q
    make_upper_triangular(nc, trimask, val=1.0, diag=True)
    growf = consts.tile([1, D], FP32)
    nc.sync.dma_start(out=growf, in_=bass.AP(tensor=D_["ln_g"], offset=0,
                                             ap=[[0, 1], [1, D]]))
    grow = consts.tile([1, D], FP32R)    # ln_g as a row (outer-product lhsT)
    nc.vector.tensor_copy(out=grow, in_=growf)
    gTc = consts.tile([P, CC], FP32)     # ln_g column layout (per-partition)
    nc.sync.dma_start(out=gTc, in_=D_["ln_g"].ap().rearrange("(cc p) -> p cc", p=P))
    bT = consts.tile([P, CC], FP32)      # ln_b column layout (per-partition)
    nc.sync.dma_start(out=bT, in_=D_["ln_b"].ap().rearrange("(cc p) -> p cc", p=P))
    onesPf = consts.tile([P, 1], FP32)
    nc.vector.memset(onesPf, 1.0)
    onesP = consts.tile([P, 1], FP32R)
    nc.vector.tensor_copy(out=onesP, in_=onesPf)
    onesrowf = consts.tile([1, P], FP32)
    nc.vector.memset(onesrowf, 1.0)
    onesrow = consts.tile([1, P], FP32R)
    nc.vector.tensor_copy(out=onesrow, in_=onesrowf)
    epst = consts.tile([1, 1], FP32)
    nc.vector.memset(epst, LN_EPS)
    idx = consts.tile([P, TOWN // P], I32)
    nc.sync.dma_start(out=idx, in_=D_["src"].ap().rearrange("(tc p) -> p tc", p=P))

    onesPr, onesrowr, growr = onesP, onesrow, grow

    # ---- layer norm on one 512-token chunk, feature-major ----
    # s [P, CC, CH] fp32 -> writes normalized values into dest(cc) slices.
    # dest_fn(cc) returns the output AP for chunk cc.
    def layer_norm_chunk(s, dest_fn):
        mean = small.tile([1, CH], FP32R, tag="lnmean", name="mean")
        msq = small.tile([1, CH], FP32, tag="lnmsq", name="msq")
        var = small.tile([1, CH], FP32, tag="lnvar", name="var")
        rstd = small.tile([1, CH], FP32R, tag="lnrstd", name="rstd")
        msrow = small.tile([1, CH], FP32R, tag="lnmsrow", name="msrow")
        ps_m = psC.tile([1, CH], FP32, tag="mm", name="ps_m")
        ps_q = psC.tile([1, CH], FP32, tag="mm", name="ps_q")
        for cc in range(CC):
            sq = scratch.tile([P, CH], FP32R, tag="scr", name="sq")
            eng = nc.vector if no_pool else nc.gpsimd
            eng.tensor_mul(out=sq, in0=s[:, cc, :], in1=s[:, cc, :])
            nc.tensor.matmul(ps_m, onesPr, s[:, cc, :],
                             start=(cc == 0), stop=(cc == CC - 1))
            nc.tensor.matmul(ps_q, onesPr, sq,
                             start=(cc == 0), stop=(cc == CC - 1))
        with nc.allow_low_precision(reason="fp32r LN mean row"):
            nc.scalar.activation(out=mean, in_=ps_m, func=Copy, scale=1.0 / D)
        nc.scalar.activation(out=msq, in_=ps_q, func=Copy, scale=1.0 / D)
        nc.vector.tensor_mul(out=var, in0=mean, in1=mean)
        nc.vector.tensor_sub(out=var, in0=msq, in1=var)
        nc.scalar.activation(out=rstd, in_=var, func=Sqrt, bias=epst, scale=1.0)
        with nc.allow_low_precision(reason="fp32r rstd feeds fp32r matmul broadcast"):
            nc.vector.reciprocal(out=rstd, in_=rstd)
        nc.vector.tensor_mul(out=msrow, in0=mean, in1=rstd)
        if ln_row:
            # 2 row-broadcasts (copied straight to SBUF to keep psum free),
            # then classic 3-op normalize per cc: moves work from the
            # saturated PE to the underused DVE.
            mBp = psC.tile([P, CH], FP32, tag="mm", name="mBp")
            nc.tensor.matmul(mBp, onesrow, mean, start=True, stop=True)
            mBs = scratch.tile([P, CH], FP32, tag="scr", name="mBs")
            nc.vector.tensor_copy(out=mBs, in_=mBp)
            rBp = psC.tile([P, CH], FP32, tag="mm", name="rBp")
            nc.tensor.matmul(rBp, onesrow, rstd, start=True, stop=True)
            rBs = scratch.tile([P, CH], FP32, tag="scr", name="rBs")
            nc.vector.tensor_copy(out=rBs, in_=rBp)
            for cc in range(CC):
                t = scratch.tile([P, CH], FP32, tag="scr", name="t")
                nc.vector.tensor_sub(out=t, in0=s[:, cc, :], in1=mBs)
                nc.vector.tensor_mul(out=t, in0=t, in1=rBs)
                nc.vector.tensor_scalar(out=dest_fn(cc), in0=t,
                                        scalar1=gTc[:, cc:cc + 1],
                                        scalar2=bT[:, cc:cc + 1],
                                        op0=mybir.AluOpType.mult, op1=ADD)
        else:
            for cc in range(CC):
                gsl = growr[0:1, cc * P:(cc + 1) * P]
                sgB = psC.tile([P, CH], FP32, tag="mm", name="sgB")
                nc.tensor.matmul(sgB, gsl, rstd, start=True, stop=True)
                gmsB = psC.tile([P, CH], FP32, tag="mm", name="gmsB")
                nc.tensor.matmul(gmsB, gsl, msrow, start=True, stop=True)
                t = scratch.tile([P, CH], FP32, tag="scr", name="t")
                nc.vector.tensor_mul(out=t, in0=s[:, cc, :], in1=sgB)
                # dest = (t + b) - g*mean*rstd
                nc.vector.scalar_tensor_tensor(
                    out=dest_fn(cc), in0=t, scalar=bT[:, cc:cc + 1], in1=gmsB,
                    op0=ADD, op1=SUB)

    # ---- embedding gather for own tokens -> x_own [P, CC, TOWN] fp32 ----
    x_own = stream.tile([P, CC, TOWN], FP32, tag="x_own")
    for c in range(NCH):
        xhbc = chk.tile([P, CC, CH], BF16, tag="xhbc", name="xhbc")
        for tb in range(CH // P):
            tcN = c * (CH // P) + tb
            rows = xstrm.tile([P, D], FP32, tag="erows", name="erows")
            nc.gpsimd.indirect_dma_start(
                out=rows, out_offset=None, in_=D_["emb"].ap(),
                in_offset=bass.IndirectOffsetOnAxis(ap=idx[:, tcN:tcN + 1], axis=0))
            for cc in range(CC):
                pt = psC.tile([P, P], FP32, tag="mm", name="pt")
                nc.tensor.transpose(pt, rows[:, cc * P:(cc + 1) * P], ident)
                sl = slice(tcN * P, (tcN + 1) * P)
                nc.vector.tensor_copy(out=x_own[:, cc, sl], in_=pt)
                eng = nc.vector if no_pool else nc.gpsimd
                eng.tensor_copy(out=xhbc[:, cc, tb * P:(tb + 1) * P],
                                in_=x_own[:, cc, sl])
        nc.sync.dma_start(
            out=D_["xh"][0].ap()[c].rearrange("(cc p) t -> p cc t", p=P),
            in_=xhbc)
        nc.gpsimd.collective_compute(
            kind="AllGather", op=mybir.AluOpType.bypass, replica_groups=GROUPS,
            ins=[D_["xh"][0].ap()[c]], outs=[D_["xf"][0].ap()[c]])

    # ---- per-layer weight loads ----
    def emit_weights_qkv(l):
        W = {}
        W["wq"] = wpool.tile([P, CC, DL], BF16, tag="wq", name="wq_t")
        nc.sync.dma_start(out=W["wq"], in_=D_["wq"].ap()[l].rearrange("(cc p) d -> p cc d", p=P))
        W["wk"] = wpool.tile([P, CC, DL], BF16, tag="wk", name="wk_t")
        nc.sync.dma_start(out=W["wk"], in_=D_["wk"].ap()[l].rearrange("(cc p) d -> p cc d", p=P))
        W["wv"] = wpool.tile([P, CC, DL], BF16, tag="wv", name="wv_t")
        nc.sync.dma_start(out=W["wv"], in_=D_["wv"].ap()[l].rearrange("(cc p) d -> p cc d", p=P))
        W["wo"] = wpool.tile([P, 2, D], BF16, tag="wo", name="wo_t")
        nc.sync.dma_start(out=W["wo"], in_=D_["wo"].ap()[l].rearrange("(hc p) d -> p hc d", p=P))
        W["bq"] = wpool.tile([P, 2], FP32, tag="bq", name="bq_t")
        nc.sync.dma_start(out=W["bq"], in_=D_["bq"].ap()[l].rearrange("(hc p) -> p hc", p=P))
        W["bk"] = wpool.tile([P, 2], FP32, tag="bk", name="bk_t")
        nc.sync.dma_start(out=W["bk"], in_=D_["bk"].ap()[l].rearrange("(hc p) -> p hc", p=P))
        W["bv"] = wpool.tile([P, DL], FP32, tag="bvB", name="bvB")
        nc.sync.dma_start(out=W["bv"], in_=bass.AP(tensor=D_["bv"], offset=l * DL,
                                                   ap=[[0, P], [1, DL]]))
        W["bo"] = wpool.tile([P, CC], FP32, tag="bo", name="bo_t")
        nc.sync.dma_start(out=W["bo"], in_=D_["bo"].ap()[l].rearrange("(cc p) -> p cc", p=P))
        return W

    def emit_weights_ffn(l, W):
        W["w1"] = wpool.tile([P, CC, F], BF16, tag="w1S", name="w1S")
        nc.sync.dma_start(out=W["w1"], in_=D_["w1"].ap()[l].rearrange("(cc p) f -> p cc f", p=P))
        W["w2"] = wpool.tile([P, FC, D], BF16, tag="w2S", name="w2S")
        nc.sync.dma_start(out=W["w2"], in_=D_["w2"].ap()[l].rearrange("(fc p) d -> p fc d", p=P))
        W["b1"] = wpool.tile([P, FC], FP32, tag="b1", name="b1_t")
        nc.sync.dma_start(out=W["b1"], in_=D_["b1"].ap()[l].rearrange("(fc p) -> p fc", p=P))
        W["b2"] = wpool.tile([P, CC], FP32, tag="b2", name="b2_t")
        nc.sync.dma_start(out=W["b2"], in_=D_["b2"].ap()[l].rearrange("(cc p) -> p cc", p=P))

    def qkv_tg(l, tg, W, T):
        if "QT" not in T:
            T["QT"] = acts.tile([P, 2, S], BF16, tag="QT", name="QT")
            T["KT"] = acts.tile([P, 2, S], BF16, tag="KT", name="KT")
            T["VR"] = acts.tile([P, S // P, HL, Dh + 1], BF16, tag="VR", name="VR")
            nc.vector.memset(T["VR"][:, :, :, Dh:Dh + 1], 1.0)
        c, r = tg % 2, tg // 2
        xbt = xstrm.tile([P, CC, CH], BF16, tag="xbt", name="xbt")
        nc.sync.dma_start(
            out=xbt,
            in_=D_["xf"][l].ap()[c, r].rearrange("(cc p) t -> p cc t", p=P))
        sl = slice(tg * CH, (tg + 1) * CH)
        for dst, w_t, b_t in ((T["QT"], W["wq"], W["bq"]), (T["KT"], W["wk"], W["bk"])):
            for hc in range(2):
                ps = psC.tile([P, CH], FP32, tag="mm", name="ps_qk")
                for cc in range(CC):
                    nc.tensor.matmul(
                        ps, w_t[:, cc, hc * P:(hc + 1) * P], xbt[:, cc, :],
                        start=(cc == 0), stop=(cc == CC - 1))
                nc.scalar.activation(out=dst[:, hc, sl], in_=ps, func=Ident,
                                     bias=b_t[:, hc:hc + 1], scale=1.0)
        for tb in range(CH // P):
            tcN = tg * (CH // P) + tb
            ps = psC.tile([P, DL], FP32, tag="mm", name="ps_v")
            for cc in range(CC):
                nc.tensor.matmul(
                    ps, xbt[:, cc, tb * P:(tb + 1) * P], W["wv"][:, cc, :],
                    start=(cc == 0), stop=(cc == CC - 1))
            nc.vector.tensor_add(
                out=T["VR"][:, tcN, :, 0:Dh],
                in0=ps.rearrange("p (h d) -> p h d", h=HL),
                in1=W["bv"].rearrange("p (h d) -> p h d", h=HL))

    def attn_head(qg, h, T):
        if "attnT" not in T:
            T["attnT"] = acts.tile([P, 2, S], BF16, tag="attnT", name="attnT")
        qsl = slice(qg * CH, (qg + 1) * CH)
        kmax = qg * 4 + 3
        hp, ho = h // 2, (h % 2) * Dh
        qt_h = T["QT"][ho:ho + Dh, hp, :]
        kt_h = T["KT"][ho:ho + Dh, hp, :]
        av = psB.tile([Dh + 1, CH], FP32, tag="av", name="av")
        if fine_attn:
            for kb in range(kmax + 1):
                koff = max(0, kb - qg * 4) * P
                sc = psA.tile([P, CH], FP32, tag="sc", name="sc")
                nc.tensor.matmul(sc[:, koff:],
                                 kt_h[:, kb * P:(kb + 1) * P],
                                 qt_h[:, qg * CH + koff:(qg + 1) * CH],
                                 start=True, stop=True)
                ex = expp.tile([P, CH], BF16, tag="ex", name="ex")
                nc.scalar.activation(out=ex[:, koff:], in_=sc[:, koff:],
                                     func=Exp, scale=1.0 / 8.0)
                dj = kb - qg * 4
                if 0 <= dj <= 3:
                    nc.vector.tensor_mul(out=ex[:, dj * P:(dj + 1) * P],
                                         in0=ex[:, dj * P:(dj + 1) * P],
                                         in1=trimask)
                nc.tensor.matmul(av[:, koff:], T["VR"][:, kb, h, :], ex[:, koff:],
                                 start=(kb == 0), stop=(kb == kmax))
        else:
            for kb0 in range(0, kmax + 1, 2):
                sc = psA.tile([P, 1024], FP32, tag="sc", name="sc")
                for j in range(2):
                    koff = max(0, kb0 + j - qg * 4) * P
                    nc.tensor.matmul(sc[:, j * CH + koff:(j + 1) * CH],
                                     kt_h[:, (kb0 + j) * P:(kb0 + j + 1) * P],
                                     qt_h[:, qg * CH + koff:(qg + 1) * CH],
                                     start=True, stop=True)
                ex = expp.tile([P, 1024], BF16, tag="ex", name="ex")
                off0 = max(0, kb0 - qg * 4) * P
                nc.scalar.activation(out=ex[:, off0:], in_=sc[:, off0:],
                                     func=Exp, scale=1.0 / 8.0)
                for j in range(2):
                    kb = kb0 + j
                    exj = ex[:, j * CH:(j + 1) * CH]
                    dj = kb - qg * 4
                    if 0 <= dj <= 3:  # diagonal block: apply causal mask
                        nc.vector.tensor_mul(out=exj[:, dj * P:(dj + 1) * P],
                                             in0=exj[:, dj * P:(dj + 1) * P],
                                             in1=trimask)
                    off = max(0, dj) * P
                    nc.tensor.matmul(av[:, off:], T["VR"][:, kb, h, :], exj[:, off:],
                                     start=(kb == 0), stop=(kb == kmax))
        # normalize: attnT = av[0:Dh] * broadcast(1/av[Dh])
        if no_pool:
            rrow = small.tile([1, CH], FP32R, tag="rrow", name="rrow")
            with nc.allow_low_precision(reason="softmax recip row"):
                nc.vector.reciprocal(out=rrow, in_=av[Dh:Dh + 1, :])
            rbt = psC.tile([Dh, CH], FP32, tag="mm", name="rbt")
            nc.tensor.matmul(rbt, onesrow[0:1, 0:Dh], rrow, start=True, stop=True)
            att_sl = T["attnT"][ho:ho + Dh, hp, qsl]
            nc.vector.tensor_copy(out=att_sl, in_=av[0:Dh, :])
            nc.vector.tensor_mul(out=att_sl, in0=att_sl, in1=rbt)
        else:
            rrow = small.tile([1, CH], FP32, tag="rrow", name="rrow")
            nc.vector.reciprocal(out=rrow, in_=av[Dh:Dh + 1, :])
            rbt = scratch.tile([Dh, CH], FP32, tag="rbt", name="rbt")
            nc.gpsimd.partition_broadcast(out_ap=rbt, in_ap=rrow)
            nc.vector.tensor_mul(out=T["attnT"][ho:ho + Dh, hp, qsl],
                                 in0=av[0:Dh, :], in1=rbt)

    def opart(l, qg, W, T):
        qsl = slice(qg * CH, (qg + 1) * CH)
        c, half = qg % 2, qg // 2
        for dc in range(CC):
            ps = psC.tile([P, CH], FP32, tag="mm", name="ps_o")
            for hc in range(2):
                nc.tensor.matmul(ps, W["wo"][:, hc, dc * P:(dc + 1) * P],
                                 T["attnT"][:, hc, qsl],
                                 start=(hc == 0), stop=(hc == 1))
            ob = scratch.tile([P, CH], BF16, tag="ob", name="ob")
            nc.vector.tensor_copy(out=ob, in_=ps)
            nc.sync.dma_start(
                out=D_["apart"][l].ap()[c, half, dc * P:(dc + 1) * P, :],
                in_=ob)

    def rs(l, c):
        nc.gpsimd.collective_compute(
            kind="ReduceScatter", op=ADD, replica_groups=GROUPS,
            ins=[D_["apart"][l].ap()[c]], outs=[D_["aown"][l].ap()[c]])

    def resid1_lna(l, c, W, T):
        if "yb" not in T:
            T["yb"] = stream.tile([P, CC, TOWN], BF16, tag="yb", name="yb")
        csl = slice(c * CH, (c + 1) * CH)
        ar = chk.tile([P, CC, CH], BF16, tag="ar", name="ar")
        nc.sync.dma_start(
            out=ar, in_=D_["aown"][l].ap()[c].rearrange("(cc p) t -> p cc t", p=P))
        s1 = chk.tile([P, CC, CH], FP32R, tag="sres", name="s1")
        for cc in range(CC):
            nc.vector.scalar_tensor_tensor(
                out=s1[:, cc, :], in0=ar[:, cc, :], scalar=W["bo"][:, cc:cc + 1],
                in1=x_own[:, cc, csl], op0=ADD, op1=ADD)
        yb = T["yb"]
        layer_norm_chunk(s1, lambda cc, _csl=csl: yb[:, cc, _csl])

    def ffn(l, c, W, T):
        csl = slice(c * CH, (c + 1) * CH)
        h1T = h1pool.tile([P, FC, CH], BF16, tag="h1T", name="h1T")
        for fc in range(FC):
            ps = psC.tile([P, CH], FP32, tag="mm", name="ps_f1")
            for cc in range(CC):
                nc.tensor.matmul(ps, W["w1"][:, cc, fc * P:(fc + 1) * P],
                                 T["yb"][:, cc, csl],
                                 start=(cc == 0), stop=(cc == CC - 1))
            nc.scalar.activation(out=h1T[:, fc, :], in_=ps, func=Relu,
                                 bias=W["b1"][:, fc:fc + 1])
        fob = chk.tile([P, CC, CH], BF16, tag="fob", name="fob")
        for dc in range(CC):
            ps = psC.tile([P, CH], FP32, tag="mm", name="ps_f2")
            for fc in range(FC):
                nc.tensor.matmul(ps, W["w2"][:, fc, dc * P:(dc + 1) * P],
                                 h1T[:, fc, :],
                                 start=(fc == 0), stop=(fc == FC - 1))
            nc.scalar.activation(out=fob[:, dc, :], in_=ps, func=Relu,
                                 bias=W["b2"][:, dc:dc + 1])
        return fob

    def resid2_lnb_ag(l, c, fob, T):
        csl = slice(c * CH, (c + 1) * CH)
        s2 = chk.tile([P, CC, CH], FP32R, tag="sres", name="s2")
        for cc in range(CC):
            nc.vector.tensor_add(out=s2[:, cc, :], in0=T["yb"][:, cc, csl],
                                 in1=fob[:, cc, :])
        layer_norm_chunk(s2, lambda cc, _csl=csl: x_own[:, cc, _csl])
        if l < L - 1:
            xhbc = chk.tile([P, CC, CH], BF16, tag="xhbc", name="xhbc")
            eng = nc.vector if no_pool else nc.gpsimd
            for cc in range(CC):
                eng.tensor_copy(out=xhbc[:, cc, :], in_=x_own[:, cc, csl])
            nc.sync.dma_start(
                out=D_["xh"][l + 1].ap()[c].rearrange("(cc p) t -> p cc t", p=P),
                in_=xhbc)
            nc.gpsimd.collective_compute(
                kind="AllGather", op=mybir.AluOpType.bypass,
                replica_groups=GROUPS,
                ins=[D_["xh"][l + 1].ap()[c]], outs=[D_["xf"][l + 1].ap()[c]])

    def out_rows(tb):
        rows = xstrm.tile([P, D], FP32, tag="orows", name="orows")
        for cc in range(CC):
            pt = psC.tile([P, P], FP32, tag="mm", name="pt_o")
            nc.tensor.transpose(pt, x_own[:, cc, tb * P:(tb + 1) * P], ident)
            nc.vector.tensor_copy(out=rows[:, cc * P:(cc + 1) * P], in_=pt)
        nc.sync.dma_start(out=D_["out"].ap()[tb * P:(tb + 1) * P, :], in_=rows)

    # ---- layer schedule (two variants, BASS_ENC_SCHED picks) ----
    sched = os.environ.get("BASS_ENC_SCHED", "v1")
    if sched == "v1":
        for l in range(L):
            W = emit_weights_qkv(l)
            T = {}
            qkv_tg(l, 0, W, T)
            qkv_tg(l, 2, W, T)
            for h in range(HL):
                attn_head(0, h, T)
            opart(l, 0, W, T)
            qkv_tg(l, 1, W, T)
            qkv_tg(l, 3, W, T)
            for h in range(HL):
                attn_head(2, h, T)
            opart(l, 2, W, T)
            rs(l, 0)
            emit_weights_ffn(l, W)
            for h in range(HL):
                attn_head(1, h, T)
            opart(l, 1, W, T)
            for h in range(HL):
                attn_head(3, h, T)
            opart(l, 3, W, T)
            rs(l, 1)
            resid1_lna(l, 0, W, T)
            fob0 = ffn(l, 0, W, T)
            resid2_lnb_ag(l, 0, fob0, T)
            resid1_lna(l, 1, W, T)
            fob1 = ffn(l, 1, W, T)
            resid2_lnb_ag(l, 1, fob1, T)
        for tb in range(TOWN // P):
            out_rows(tb)
    else:
        W = emit_weights_qkv(0)
        T = {}
        qkv_tg(0, 0, W, T)
        qkv_tg(0, 2, W, T)
        for h in range(HL):
            attn_head(0, h, T)
        opart(0, 0, W, T)
        for l in range(L):
            last = l == L - 1
            emit_weights_ffn(l, W)
            qkv_tg(l, 1, W, T)
            qkv_tg(l, 3, W, T)
            for h in range(HL):
                attn_head(2, h, T)
            opart(l, 2, W, T)
            rs(l, 0)
            for h in range(HL):
                attn_head(3, h, T)
            opart(l, 3, W, T)
            resid1_lna(l, 0, W, T)
            fob0 = ffn(l, 0, W, T)
            resid2_lnb_ag(l, 0, fob0, T)
            for h in range(HL):
                attn_head(1, h, T)
            opart(l, 1, W, T)
            rs(l, 1)
            if not last:
                Wn = emit_weights_qkv(l + 1)
                Tn = {}
                qkv_tg(l + 1, 0, Wn, Tn)
                qkv_tg(l + 1, 2, Wn, Tn)
                attn_head(0, 0, Tn)
                attn_head(0, 1, Tn)
            else:
                for tb in range(TOWN // (2 * P)):
                    out_rows(tb)
            resid1_lna(l, 1, W, T)
            fob1 = ffn(l, 1, W, T)
            resid2_lnb_ag(l, 1, fob1, T)
            if not last:
                attn_head(0, 2, Tn)
                attn_head(0, 3, Tn)
                opart(l + 1, 0, Wn, Tn)
                W, T = Wn, Tn
        for tb in range(TOWN // (2 * P), TOWN // P):
            out_rows(tb)

    ctx.close()


def _get_program():
    no_cc = bool(int(os.environ.get("BASS_ENC_NOCC", "0")))
    key = ("nc", no_cc)
    if key not in _CACHED:
        _CACHED[key] = _build_program(no_cc)
    return _CACHED[key]


def prep_in_maps(inputs):
    def f32(x):
        return np.ascontiguousarray(np.asarray(x, dtype=np.float32))

    def bf(x):
        return np.ascontiguousarray(np.asarray(x, dtype=np.float32).astype(ml_dtypes.bfloat16))

    source = np.asarray(inputs["source"]).astype(np.int32)
    emb = f32(inputs["emb"])
    ln_g, ln_b = f32(inputs["ln_g"]), f32(inputs["ln_b"])
    w1a, b1a = bf(inputs["w1"]), f32(inputs["b1"])
    w2a, b2a = bf(inputs["w2"]), f32(inputs["b2"])
    wqa, wka, wva = np.asarray(inputs["wq"]), np.asarray(inputs["wk"]), np.asarray(inputs["wv"])
    bqa, bka, bva = np.asarray(inputs["bq"]), np.asarray(inputs["bk"]), np.asarray(inputs["bv"])
    woa, boa = np.asarray(inputs["wo"]), f32(inputs["bo"])

    in_maps = []
    for core in range(8):
        b, half = core // 2, core % 2
        hsl = slice(half * DL, (half + 1) * DL)
        in_maps.append({
            "src": np.ascontiguousarray(source[b, half * TOWN:(half + 1) * TOWN]),
            "emb": emb,
            "wq": bf(wqa[:, :, hsl]), "wk": bf(wka[:, :, hsl]), "wv": bf(wva[:, :, hsl]),
            "bq": f32(bqa[:, hsl]), "bk": f32(bka[:, hsl]), "bv": f32(bva[:, hsl]),
            "wo": bf(woa[:, hsl, :]), "bo": boa,
            "w1": w1a, "b1": b1a, "w2": w2a, "b2": b2a,
            "ln_g": ln_g, "ln_b": ln_b,
        })
    return in_maps


def kernel(**inputs):
    nc = _get_program()
    in_maps = prep_in_maps(inputs)
    trace = bool(int(os.environ.get("BASS_ENC_TRACE", "0")))
    res = bass_utils.run_bass_kernel_spmd(nc, in_maps, core_ids=list(range(8)),
                                          trace=trace)
    _CACHED["last_results"] = res

    outp = np.empty((B, S, D), np.float32)
    for core in range(8):
        b, half = core // 2, core % 2
        outp[b, half * TOWN:(half + 1) * TOWN, :] = res.results[core]["out"]
    return outp


# revision 39
# speedup vs baseline: 1.0385x; 1.0385x over previous
"""Trainium2 Bass kernel for nn_Encoder (6-layer causal transformer encoder).

Sharding: 8 cores = 4 batch elements x 2-core tensor-parallel pairs.
Within a pair: attention is head-split (4 of 8 heads per core), FFN/LN/residual
are token-split (1024 of 2048 tokens per core).  Rank asymmetry is expressed
purely through ReduceScatter / AllGather rank order, so the SPMD program is
identical on every core.

v2 changes vs baseline:
 - collectives split into 2 token-chunks and reordered so they overlap
   compute (AG chunk issued right after its LN-B chunk; RS chunk issued
   after the two q-groups that feed it, overlapping the other two).
 - LayerNorm + softmax normalization broadcast via PE rank-1 outer products
   instead of DRAM round-trips.
 - w1/w2 SBUF-resident per layer (single load, no streaming stalls).
 - LN statistics matmuls run on float32r data at full PE rate (no bf16 copy).
 - score matmuls restricted to causally-valid columns in the diagonal band.
 - HW is PE-bound at the ~1.2GHz p-state, so the schedule keeps matmul
   streams dense and avoids GPSIMD software ops (slower on HW than modeled).
"""

import os
import sys

sys.path.insert(0, "/opt/trn_rl_repo")

import numpy as np
import ml_dtypes

import concourse.bass as bass
import concourse.mybir as mybir
import concourse.tile as tile
from concourse import bacc, bass_utils
from concourse.masks import make_identity, make_upper_triangular

# Problem constants (hardcoded per harness contract).
B, S, V, D, F, L = 4, 2048, 32000, 512, 2048, 6
H, Dh = 8, 64
HL = H // 2            # local heads per core (4)
DL = HL * Dh           # 256 local head-dims
TOWN = S // 2          # 1024 tokens owned per core
P = 128
CC = D // P            # 4 c-chunks
FC = F // P            # 16 f-chunks
CH = 512               # token chunk for collectives / LN / FFN
NCH = TOWN // CH       # 2 chunks
LN_EPS = 1e-5

FP32 = mybir.dt.float32
FP32R = mybir.dt.float32r
BF16 = mybir.dt.bfloat16
I32 = mybir.dt.int32

GROUPS = [[0, 1], [2, 3], [4, 5], [6, 7]]

_CACHED = {}


def _build_program(no_cc=False):
    nc = bacc.Bacc("TRN2", target_bir_lowering=False, debug=False, num_devices=8)
    if no_cc:
        # benchmarking variant: collectives replaced by a local DRAM copy
        # (wrong results; identical compute/DMA structure)
        def fake_cc(kind, op, replica_groups, ins, outs, **kw):
            src = ins[0]
            dst = outs[0]
            n = min(src.size(), dst.size())
            nc.sync.dma_start(
                out=bass.AP(tensor=dst.tensor, offset=dst.offset, ap=[[1, n]]),
                in_=bass.AP(tensor=src.tensor, offset=src.offset, ap=[[1, n]]))

        nc.gpsimd.collective_compute = fake_cc

    D_ = {}
    D_["src"] = nc.dram_tensor("src", [TOWN], I32, kind="ExternalInput")
    D_["emb"] = nc.dram_tensor("emb", [V, D], FP32, kind="ExternalInput")
    D_["wq"] = nc.dram_tensor("wq", [L, D, DL], BF16, kind="ExternalInput")
    D_["wk"] = nc.dram_tensor("wk", [L, D, DL], BF16, kind="ExternalInput")
    D_["wv"] = nc.dram_tensor("wv", [L, D, DL], BF16, kind="ExternalInput")
    D_["wo"] = nc.dram_tensor("wo", [L, DL, D], BF16, kind="ExternalInput")
    D_["bq"] = nc.dram_tensor("bq", [L, DL], FP32, kind="ExternalInput")
    D_["bk"] = nc.dram_tensor("bk", [L, DL], FP32, kind="ExternalInput")
    D_["bv"] = nc.dram_tensor("bv", [L, DL], FP32, kind="ExternalInput")
    D_["bo"] = nc.dram_tensor("bo", [L, D], FP32, kind="ExternalInput")
    D_["w1"] = nc.dram_tensor("w1", [L, D, F], BF16, kind="ExternalInput")
    D_["b1"] = nc.dram_tensor("b1", [L, F], FP32, kind="ExternalInput")
    D_["w2"] = nc.dram_tensor("w2", [L, F, D], BF16, kind="ExternalInput")
    D_["b2"] = nc.dram_tensor("b2", [L, D], FP32, kind="ExternalInput")
    D_["ln_g"] = nc.dram_tensor("ln_g", [D], FP32, kind="ExternalInput")
    D_["ln_b"] = nc.dram_tensor("ln_b", [D], FP32, kind="ExternalInput")
    D_["out"] = nc.dram_tensor("out", [TOWN, D], FP32, kind="ExternalOutput")

    # DRAM scratch, chunk-major so each collective chunk is contiguous.
    D_["xh"] = [nc.dram_tensor(f"xh{l}", [NCH, D, CH], BF16, kind="Internal")
                for l in range(L)]
    D_["xf"] = [nc.dram_tensor(f"xf{l}", [NCH, 2, D, CH], BF16, kind="Internal")
                for l in range(L)]
    D_["apart"] = [nc.dram_tensor(f"apart{l}", [NCH, 2, D, CH], BF16,
                                  kind="Internal")
                   for l in range(L)]
    D_["aown"] = [nc.dram_tensor(f"aown{l}", [NCH, D, CH], BF16, kind="Internal")
                  for l in range(L)]

    with tile.TileContext(nc) as tc:
        _emit(nc, tc, D_)

    nc.compile()
    return nc


def _emit(nc, tc, D_):
    from contextlib import ExitStack

    no_pool = bool(int(os.environ.get("BASS_ENC_NOPOOL", "1")))
    fine_attn = bool(int(os.environ.get("BASS_ENC_FINEATTN", "0")))
    ln_row = False  # row-broadcast LN variant deadlocked the tile scheduler

    ctx = ExitStack()
    Exp = mybir.ActivationFunctionType.Exp
    Relu = mybir.ActivationFunctionType.Relu
    Copy = mybir.ActivationFunctionType.Copy
    Ident = mybir.ActivationFunctionType.Identity
    Sqrt = mybir.ActivationFunctionType.Sqrt
    ADD = mybir.AluOpType.add
    SUB = mybir.AluOpType.subtract

    consts = ctx.enter_context(tc.tile_pool(name="consts", bufs=1))
    wpool = ctx.enter_context(tc.tile_pool(name="weights", bufs=1))
    stream = ctx.enter_context(tc.tile_pool(name="stream", bufs=1))
    acts = ctx.enter_context(tc.tile_pool(name="acts", bufs=1))
    xstrm = ctx.enter_context(tc.tile_pool(name="xstrm", bufs=2))
    chk = ctx.enter_context(tc.tile_pool(name="chunks", bufs=2))
    small = ctx.enter_context(tc.tile_pool(name="small", bufs=1))
    scratch = ctx.enter_context(tc.tile_pool(name="scratch", bufs=2))
    h1pool = ctx.enter_context(tc.tile_pool(name="h1pool", bufs=1))
    expp = ctx.enter_context(tc.tile_pool(name="exp", bufs=4))
    psA = ctx.enter_context(tc.tile_pool(
        name="psA",
        bufs=4 if bool(int(os.environ.get("BASS_ENC_FINEATTN", "0"))) else 2,
        space="PSUM"))
    psB = ctx.enter_context(tc.tile_pool(name="psB", bufs=2, space="PSUM"))
    psC = ctx.enter_context(tc.tile_pool(name="psC", bufs=2, space="PSUM"))

    # ---- constants ----
    ident = consts.tile([P, P], FP32)
    make_identity(nc, ident)
    trimask = consts.tile([P, P], BF16)  # 1 where k<=q
    make_upper_triangular(nc, trimask, val=1.0, diag=True)
    growf = consts.tile([1, D], FP32)
    nc.sync.dma_start(out=growf, in_=bass.AP(tensor=D_["ln_g"], offset=0,
                                             ap=[[0, 1], [1, D]]))
    grow = consts.tile([1, D], FP32R)    # ln_g as a row (outer-product lhsT)
    nc.vector.tensor_copy(out=grow, in_=growf)
    gTc = consts.tile([P, CC], FP32)     # ln_g column layout (per-partition)
    nc.sync.dma_start(out=gTc, in_=D_["ln_g"].ap().rearrange("(cc p) -> p cc", p=P))
    bT = consts.tile([P, CC], FP32)      # ln_b column layout (per-partition)
    nc.sync.dma_start(out=bT, in_=D_["ln_b"].ap().rearrange("(cc p) -> p cc", p=P))
    onesPf = consts.tile([P, 1], FP32)
    nc.vector.memset(onesPf, 1.0)
    onesP = consts.tile([P, 1], FP32R)
    nc.vector.tensor_copy(out=onesP, in_=onesPf)
    onesrowf = consts.tile([1, P], FP32)
    nc.vector.memset(onesrowf, 1.0)
    onesrow = consts.tile([1, P], FP32R)
    nc.vector.tensor_copy(out=onesrow, in_=onesrowf)
    epst = consts.tile([1, 1], FP32)
    nc.vector.memset(epst, LN_EPS)
    idx = consts.tile([P, TOWN // P], I32)
    nc.sync.dma_start(out=idx, in_=D_["src"].ap().rearrange("(tc p) -> p tc", p=P))

    onesPr, onesrowr, growr = onesP, onesrow, grow

    # ---- layer norm on one 512-token chunk, feature-major ----
    # s [P, CC, CH] fp32 -> writes normalized values into dest(cc) slices.
    # dest_fn(cc) returns the output AP for chunk cc.
    def layer_norm_chunk(s, dest_fn):
        mean = small.tile([1, CH], FP32, tag="lnmean", name="mean")
        msq = small.tile([1, CH], FP32, tag="lnmsq", name="msq")
        var = small.tile([1, CH], FP32, tag="lnvar", name="var")
        rstd = small.tile([1, CH], FP32R, tag="lnrstd", name="rstd")
        msrow = small.tile([1, CH], FP32R, tag="lnmsrow", name="msrow")
        ps_m = psC.tile([1, CH], FP32, tag="mm", name="ps_m")
        ps_q = psC.tile([1, CH], FP32, tag="mm", name="ps_q")
        for cc in range(CC):
            sq = scratch.tile([P, CH], FP32R, tag="scr", name="sq")
            eng = nc.vector if no_pool else nc.gpsimd
            eng.tensor_mul(out=sq, in0=s[:, cc, :], in1=s[:, cc, :])
            nc.tensor.matmul(ps_m, onesPr, s[:, cc, :],
                             start=(cc == 0), stop=(cc == CC - 1))
            nc.tensor.matmul(ps_q, onesPr, sq,
                             start=(cc == 0), stop=(cc == CC - 1))
        nc.scalar.activation(out=mean, in_=ps_m, func=Copy, scale=1.0 / D)
        nc.scalar.activation(out=msq, in_=ps_q, func=Copy, scale=1.0 / D)
        nc.vector.tensor_mul(out=var, in0=mean, in1=mean)
        nc.vector.tensor_sub(out=var, in0=msq, in1=var)
        nc.scalar.activation(out=rstd, in_=var, func=Sqrt, bias=epst, scale=1.0)
        with nc.allow_low_precision(reason="fp32r rstd feeds fp32r matmul broadcast"):
            nc.vector.reciprocal(out=rstd, in_=rstd)
        nc.vector.tensor_mul(out=msrow, in0=mean, in1=rstd)
        if ln_row:
            # 2 row-broadcasts (copied straight to SBUF to keep psum free),
            # then classic 3-op normalize per cc: moves work from the
            # saturated PE to the underused DVE.
            mBp = psC.tile([P, CH], FP32, tag="mm", name="mBp")
            nc.tensor.matmul(mBp, onesrow, mean, start=True, stop=True)
            mBs = scratch.tile([P, CH], FP32, tag="scr", name="mBs")
            nc.vector.tensor_copy(out=mBs, in_=mBp)
            rBp = psC.tile([P, CH], FP32, tag="mm", name="rBp")
            nc.tensor.matmul(rBp, onesrow, rstd, start=True, stop=True)
            rBs = scratch.tile([P, CH], FP32, tag="scr", name="rBs")
            nc.vector.tensor_copy(out=rBs, in_=rBp)
            for cc in range(CC):
                t = scratch.tile([P, CH], FP32, tag="scr", name="t")
                nc.vector.tensor_sub(out=t, in0=s[:, cc, :], in1=mBs)
                nc.vector.tensor_mul(out=t, in0=t, in1=rBs)
                nc.vector.tensor_scalar(out=dest_fn(cc), in0=t,
                                        scalar1=gTc[:, cc:cc + 1],
                                        scalar2=bT[:, cc:cc + 1],
                                        op0=mybir.AluOpType.mult, op1=ADD)
        else:
            for cc in range(CC):
                gsl = growr[0:1, cc * P:(cc + 1) * P]
                sgB = psC.tile([P, CH], FP32, tag="mm", name="sgB")
                nc.tensor.matmul(sgB, gsl, rstd, start=True, stop=True)
                gmsB = psC.tile([P, CH], FP32, tag="mm", name="gmsB")
                nc.tensor.matmul(gmsB, gsl, msrow, start=True, stop=True)
                t = scratch.tile([P, CH], FP32, tag="scr", name="t")
                nc.vector.tensor_mul(out=t, in0=s[:, cc, :], in1=sgB)
                # dest = (t + b) - g*mean*rstd
                nc.vector.scalar_tensor_tensor(
                    out=dest_fn(cc), in0=t, scalar=bT[:, cc:cc + 1], in1=gmsB,
                    op0=ADD, op1=SUB)

    # ---- embedding gather for own tokens -> x_own [P, CC, TOWN] fp32 ----
    x_own = stream.tile([P, CC, TOWN], FP32, tag="x_own")
    for c in range(NCH):
        xhbc = chk.tile([P, CC, CH], BF16, tag="xhbc", name="xhbc")
        for tb in range(CH // P):
            tcN = c * (CH // P) + tb
            rows = xstrm.tile([P, D], FP32, tag="erows", name="erows")
            nc.gpsimd.indirect_dma_start(
                out=rows, out_offset=None, in_=D_["emb"].ap(),
                in_offset=bass.IndirectOffsetOnAxis(ap=idx[:, tcN:tcN + 1], axis=0))
            for cc in range(CC):
                pt = psC.tile([P, P], FP32, tag="mm", name="pt")
                nc.tensor.transpose(pt, rows[:, cc * P:(cc + 1) * P], ident)
                sl = slice(tcN * P, (tcN + 1) * P)
                nc.vector.tensor_copy(out=x_own[:, cc, sl], in_=pt)
                eng = nc.vector if no_pool else nc.gpsimd
                eng.tensor_copy(out=xhbc[:, cc, tb * P:(tb + 1) * P],
                                in_=x_own[:, cc, sl])
        nc.sync.dma_start(
            out=D_["xh"][0].ap()[c].rearrange("(cc p) t -> p cc t", p=P),
            in_=xhbc)
        nc.gpsimd.collective_compute(
            kind="AllGather", op=mybir.AluOpType.bypass, replica_groups=GROUPS,
            ins=[D_["xh"][0].ap()[c]], outs=[D_["xf"][0].ap()[c]])

    # ---- per-layer weight loads ----
    def emit_weights_qkv(l):
        W = {}
        W["wq"] = wpool.tile([P, CC, DL], BF16, tag="wq", name="wq_t")
        nc.sync.dma_start(out=W["wq"], in_=D_["wq"].ap()[l].rearrange("(cc p) d -> p cc d", p=P))
        W["wk"] = wpool.tile([P, CC, DL], BF16, tag="wk", name="wk_t")
        nc.sync.dma_start(out=W["wk"], in_=D_["wk"].ap()[l].rearrange("(cc p) d -> p cc d", p=P))
        W["wv"] = wpool.tile([P, CC, DL], BF16, tag="wv", name="wv_t")
        nc.sync.dma_start(out=W["wv"], in_=D_["wv"].ap()[l].rearrange("(cc p) d -> p cc d", p=P))
        W["wo"] = wpool.tile([P, 2, D], BF16, tag="wo", name="wo_t")
        nc.sync.dma_start(out=W["wo"], in_=D_["wo"].ap()[l].rearrange("(hc p) d -> p hc d", p=P))
        W["bq"] = wpool.tile([P, 2], FP32, tag="bq", name="bq_t")
        nc.sync.dma_start(out=W["bq"], in_=D_["bq"].ap()[l].rearrange("(hc p) -> p hc", p=P))
        W["bk"] = wpool.tile([P, 2], FP32, tag="bk", name="bk_t")
        nc.sync.dma_start(out=W["bk"], in_=D_["bk"].ap()[l].rearrange("(hc p) -> p hc", p=P))
        W["bv"] = wpool.tile([P, DL], FP32, tag="bvB", name="bvB")
        nc.sync.dma_start(out=W["bv"], in_=bass.AP(tensor=D_["bv"], offset=l * DL,
                                                   ap=[[0, P], [1, DL]]))
        W["bo"] = wpool.tile([P, CC], FP32, tag="bo", name="bo_t")
        nc.sync.dma_start(out=W["bo"], in_=D_["bo"].ap()[l].rearrange("(cc p) -> p cc", p=P))
        return W

    def emit_weights_ffn(l, W):
        W["w1"] = wpool.tile([P, CC, F], BF16, tag="w1S", name="w1S")
        nc.sync.dma_start(out=W["w1"], in_=D_["w1"].ap()[l].rearrange("(cc p) f -> p cc f", p=P))
        W["w2"] = wpool.tile([P, FC, D], BF16, tag="w2S", name="w2S")
        nc.sync.dma_start(out=W["w2"], in_=D_["w2"].ap()[l].rearrange("(fc p) d -> p fc d", p=P))
        W["b1"] = wpool.tile([P, FC], FP32, tag="b1", name="b1_t")
        nc.sync.dma_start(out=W["b1"], in_=D_["b1"].ap()[l].rearrange("(fc p) -> p fc", p=P))
        W["b2"] = wpool.tile([P, CC], FP32, tag="b2", name="b2_t")
        nc.sync.dma_start(out=W["b2"], in_=D_["b2"].ap()[l].rearrange("(cc p) -> p cc", p=P))

    def qkv_tg(l, tg, W, T):
        if "QT" not in T:
            T["QT"] = acts.tile([P, 2, S], BF16, tag="QT", name="QT")
            T["KT"] = acts.tile([P, 2, S], BF16, tag="KT", name="KT")
            T["VR"] = acts.tile([P, S // P, HL, Dh + 1], BF16, tag="VR", name="VR")
            nc.vector.memset(T["VR"][:, :, :, Dh:Dh + 1], 1.0)
        c, r = tg % 2, tg // 2
        xbt = xstrm.tile([P, CC, CH], BF16, tag="xbt", name="xbt")
        nc.sync.dma_start(
            out=xbt,
            in_=D_["xf"][l].ap()[c, r].rearrange("(cc p) t -> p cc t", p=P))
        sl = slice(tg * CH, (tg + 1) * CH)
        for dst, w_t, b_t in ((T["QT"], W["wq"], W["bq"]), (T["KT"], W["wk"], W["bk"])):
            for hc in range(2):
                ps = psC.tile([P, CH], FP32, tag="mm", name="ps_qk")
                for cc in range(CC):
                    nc.tensor.matmul(
                        ps, w_t[:, cc, hc * P:(hc + 1) * P], xbt[:, cc, :],
                        start=(cc == 0), stop=(cc == CC - 1))
                nc.scalar.activation(out=dst[:, hc, sl], in_=ps, func=Ident,
                                     bias=b_t[:, hc:hc + 1], scale=1.0)
        for tb in range(CH // P):
            tcN = tg * (CH // P) + tb
            ps = psC.tile([P, DL], FP32, tag="mm", name="ps_v")
            for cc in range(CC):
                nc.tensor.matmul(
                    ps, xbt[:, cc, tb * P:(tb + 1) * P], W["wv"][:, cc, :],
                    start=(cc == 0), stop=(cc == CC - 1))
            nc.vector.tensor_add(
                out=T["VR"][:, tcN, :, 0:Dh],
                in0=ps.rearrange("p (h d) -> p h d", h=HL),
                in1=W["bv"].rearrange("p (h d) -> p h d", h=HL))

    def attn_head(qg, h, T):
        if "attnT" not in T:
            T["attnT"] = acts.tile([P, 2, S], BF16, tag="attnT", name="attnT")
        qsl = slice(qg * CH, (qg + 1) * CH)
        kmax = qg * 4 + 3
        hp, ho = h // 2, (h % 2) * Dh
        qt_h = T["QT"][ho:ho + Dh, hp, :]
        kt_h = T["KT"][ho:ho + Dh, hp, :]
        av = psB.tile([Dh + 1, CH], FP32, tag="av", name="av")
        if fine_attn:
            for kb in range(kmax + 1):
                koff = max(0, kb - qg * 4) * P
                sc = psA.tile([P, CH], FP32, tag="sc", name="sc")
                nc.tensor.matmul(sc[:, koff:],
                                 kt_h[:, kb * P:(kb + 1) * P],
                                 qt_h[:, qg * CH + koff:(qg + 1) * CH],
                                 start=True, stop=True)
                ex = expp.tile([P, CH], BF16, tag="ex", name="ex")
                nc.scalar.activation(out=ex[:, koff:], in_=sc[:, koff:],
                                     func=Exp, scale=1.0 / 8.0)
                dj = kb - qg * 4
                if 0 <= dj <= 3:
                    nc.vector.tensor_mul(out=ex[:, dj * P:(dj + 1) * P],
                                         in0=ex[:, dj * P:(dj + 1) * P],
                                         in1=trimask)
                nc.tensor.matmul(av[:, koff:], T["VR"][:, kb, h, :], ex[:, koff:],
                                 start=(kb == 0), stop=(kb == kmax))
        else:
            for kb0 in range(0, kmax + 1, 2):
                sc = psA.tile([P, 1024], FP32, tag="sc", name="sc")
                for j in range(2):
                    koff = max(0, kb0 + j - qg * 4) * P
                    nc.tensor.matmul(sc[:, j * CH + koff:(j + 1) * CH],
                                     kt_h[:, (kb0 + j) * P:(kb0 + j + 1) * P],
                                     qt_h[:, qg * CH + koff:(qg + 1) * CH],
                                     start=True, stop=True)
                ex = expp.tile([P, 1024], BF16, tag="ex", name="ex")
                off0 = max(0, kb0 - qg * 4) * P
                nc.scalar.activation(out=ex[:, off0:], in_=sc[:, off0:],
                                     func=Exp, scale=1.0 / 8.0)
                for j in range(2):
                    kb = kb0 + j
                    exj = ex[:, j * CH:(j + 1) * CH]
                    dj = kb - qg * 4
                    if 0 <= dj <= 3:  # diagonal block: apply causal mask
                        nc.vector.tensor_mul(out=exj[:, dj * P:(dj + 1) * P],
                                             in0=exj[:, dj * P:(dj + 1) * P],
                                             in1=trimask)
                    off = max(0, dj) * P
                    nc.tensor.matmul(av[:, off:], T["VR"][:, kb, h, :], exj[:, off:],
                                     start=(kb == 0), stop=(kb == kmax))
        # normalize: attnT = av[0:Dh] * broadcast(1/av[Dh])
        if no_pool:
            rrow = small.tile([1, CH], FP32R, tag="rrow", name="rrow")
            with nc.allow_low_precision(reason="softmax recip row"):
                nc.vector.reciprocal(out=rrow, in_=av[Dh:Dh + 1, :])
            rbt = psC.tile([Dh, CH], FP32, tag="mm", name="rbt")
            nc.tensor.matmul(rbt, onesrow[0:1, 0:Dh], rrow, start=True, stop=True)
            att_sl = T["attnT"][ho:ho + Dh, hp, qsl]
            nc.vector.tensor_copy(out=att_sl, in_=av[0:Dh, :])
            nc.vector.tensor_mul(out=att_sl, in0=att_sl, in1=rbt)
        else:
            rrow = small.tile([1, CH], FP32, tag="rrow", name="rrow")
            nc.vector.reciprocal(out=rrow, in_=av[Dh:Dh + 1, :])
            rbt = scratch.tile([Dh, CH], FP32, tag="rbt", name="rbt")
            nc.gpsimd.partition_broadcast(out_ap=rbt, in_ap=rrow)
            nc.vector.tensor_mul(out=T["attnT"][ho:ho + Dh, hp, qsl],
                                 in0=av[0:Dh, :], in1=rbt)

    def opart(l, qg, W, T):
        qsl = slice(qg * CH, (qg + 1) * CH)
        c, half = qg % 2, qg // 2
        for dc in range(CC):
            ps = psC.tile([P, CH], FP32, tag="mm", name="ps_o")
            for hc in range(2):
                nc.tensor.matmul(ps, W["wo"][:, hc, dc * P:(dc + 1) * P],
                                 T["attnT"][:, hc, qsl],
                                 start=(hc == 0), stop=(hc == 1))
            ob = scratch.tile([P, CH], BF16, tag="ob", name="ob")
            nc.vector.tensor_copy(out=ob, in_=ps)
            nc.sync.dma_start(
                out=D_["apart"][l].ap()[c, half, dc * P:(dc + 1) * P, :],
                in_=ob)

    def rs(l, c):
        nc.gpsimd.collective_compute(
            kind="ReduceScatter", op=ADD, replica_groups=GROUPS,
            ins=[D_["apart"][l].ap()[c]], outs=[D_["aown"][l].ap()[c]])

    def resid1_lna(l, c, W, T):
        if "yb" not in T:
            T["yb"] = stream.tile([P, CC, TOWN], BF16, tag="yb", name="yb")
        csl = slice(c * CH, (c + 1) * CH)
        ar = chk.tile([P, CC, CH], BF16, tag="ar", name="ar")
        nc.sync.dma_start(
            out=ar, in_=D_["aown"][l].ap()[c].rearrange("(cc p) t -> p cc t", p=P))
        s1 = chk.tile([P, CC, CH], FP32R, tag="sres", name="s1")
        for cc in range(CC):
            nc.vector.scalar_tensor_tensor(
                out=s1[:, cc, :], in0=ar[:, cc, :], scalar=W["bo"][:, cc:cc + 1],
                in1=x_own[:, cc, csl], op0=ADD, op1=ADD)
        yb = T["yb"]
        layer_norm_chunk(s1, lambda cc, _csl=csl: yb[:, cc, _csl])

    def ffn(l, c, W, T):
        csl = slice(c * CH, (c + 1) * CH)
        h1T = h1pool.tile([P, FC, CH], BF16, tag="h1T", name="h1T")
        for fc in range(FC):
            ps = psC.tile([P, CH], FP32, tag="mm", name="ps_f1")
            for cc in range(CC):
                nc.tensor.matmul(ps, W["w1"][:, cc, fc * P:(fc + 1) * P],
                                 T["yb"][:, cc, csl],
                                 start=(cc == 0), stop=(cc == CC - 1))
            nc.scalar.activation(out=h1T[:, fc, :], in_=ps, func=Relu,
                                 bias=W["b1"][:, fc:fc + 1])
        fob = chk.tile([P, CC, CH], BF16, tag="fob", name="fob")
        for dc in range(CC):
            ps = psC.tile([P, CH], FP32, tag="mm", name="ps_f2")
            for fc in range(FC):
                nc.tensor.matmul(ps, W["w2"][:, fc, dc * P:(dc + 1) * P],
                                 h1T[:, fc, :],
                                 start=(fc == 0), stop=(fc == FC - 1))
            nc.scalar.activation(out=fob[:, dc, :], in_=ps, func=Relu,
                                 bias=W["b2"][:, dc:dc + 1])
        return fob

    def resid2_lnb_ag(l, c, fob, T):
        csl = slice(c * CH, (c + 1) * CH)
        s2 = chk.tile([P, CC, CH], FP32R, tag="sres", name="s2")
        for cc in range(CC):
            nc.vector.tensor_add(out=s2[:, cc, :], in0=T["yb"][:, cc, csl],
                                 in1=fob[:, cc, :])
        layer_norm_chunk(s2, lambda cc, _csl=csl: x_own[:, cc, _csl])
        if l < L - 1:
            xhbc = chk.tile([P, CC, CH], BF16, tag="xhbc", name="xhbc")
            eng = nc.vector if no_pool else nc.gpsimd
            for cc in range(CC):
                eng.tensor_copy(out=xhbc[:, cc, :], in_=x_own[:, cc, csl])
            nc.sync.dma_start(
                out=D_["xh"][l + 1].ap()[c].rearrange("(cc p) t -> p cc t", p=P),
                in_=xhbc)
            nc.gpsimd.collective_compute(
                kind="AllGather", op=mybir.AluOpType.bypass,
                replica_groups=GROUPS,
                ins=[D_["xh"][l + 1].ap()[c]], outs=[D_["xf"][l + 1].ap()[c]])

    def out_rows(tb):
        rows = xstrm.tile([P, D], FP32, tag="orows", name="orows")
        for cc in range(CC):
            pt = psC.tile([P, P], FP32, tag="mm", name="pt_o")
            nc.tensor.transpose(pt, x_own[:, cc, tb * P:(tb + 1) * P], ident)
            nc.vector.tensor_copy(out=rows[:, cc * P:(cc + 1) * P], in_=pt)
        nc.sync.dma_start(out=D_["out"].ap()[tb * P:(tb + 1) * P, :], in_=rows)

    # ---- layer schedule (two variants, BASS_ENC_SCHED picks) ----
    sched = os.environ.get("BASS_ENC_SCHED", "v1")
    if sched == "v1":
        for l in range(L):
            W = emit_weights_qkv(l)
            T = {}
            qkv_tg(l, 0, W, T)
            qkv_tg(l, 2, W, T)
            for h in range(HL):
                attn_head(0, h, T)
            opart(l, 0, W, T)
            qkv_tg(l, 1, W, T)
            qkv_tg(l, 3, W, T)
            for h in range(HL):
                attn_head(2, h, T)
            opart(l, 2, W, T)
            rs(l, 0)
            emit_weights_ffn(l, W)
            for h in range(HL):
                attn_head(1, h, T)
            opart(l, 1, W, T)
            for h in range(HL):
                attn_head(3, h, T)
            opart(l, 3, W, T)
            rs(l, 1)
            resid1_lna(l, 0, W, T)
            fob0 = ffn(l, 0, W, T)
            resid2_lnb_ag(l, 0, fob0, T)
            resid1_lna(l, 1, W, T)
            fob1 = ffn(l, 1, W, T)
            resid2_lnb_ag(l, 1, fob1, T)
        for tb in range(TOWN // P):
            out_rows(tb)
    else:
        W = emit_weights_qkv(0)
        T = {}
        qkv_tg(0, 0, W, T)
        qkv_tg(0, 2, W, T)
        for h in range(HL):
            attn_head(0, h, T)
        opart(0, 0, W, T)
        for l in range(L):
            last = l == L - 1
            emit_weights_ffn(l, W)
            qkv_tg(l, 1, W, T)
            qkv_tg(l, 3, W, T)
            for h in range(HL):
                attn_head(2, h, T)
            opart(l, 2, W, T)
            rs(l, 0)
            for h in range(HL):
                attn_head(3, h, T)
            opart(l, 3, W, T)
            resid1_lna(l, 0, W, T)
            fob0 = ffn(l, 0, W, T)
            resid2_lnb_ag(l, 0, fob0, T)
            for h in range(HL):
                attn_head(1, h, T)
            opart(l, 1, W, T)
            rs(l, 1)
            if not last:
                Wn = emit_weights_qkv(l + 1)
                Tn = {}
                qkv_tg(l + 1, 0, Wn, Tn)
                qkv_tg(l + 1, 2, Wn, Tn)
                attn_head(0, 0, Tn)
                attn_head(0, 1, Tn)
            else:
                for tb in range(TOWN // (2 * P)):
                    out_rows(tb)
            resid1_lna(l, 1, W, T)
            fob1 = ffn(l, 1, W, T)
            resid2_lnb_ag(l, 1, fob1, T)
            if not last:
                attn_head(0, 2, Tn)
                attn_head(0, 3, Tn)
                opart(l + 1, 0, Wn, Tn)
                W, T = Wn, Tn
        for tb in range(TOWN // (2 * P), TOWN // P):
            out_rows(tb)

    ctx.close()


def _get_program():
    no_cc = bool(int(os.environ.get("BASS_ENC_NOCC", "0")))
    key = ("nc", no_cc)
    if key not in _CACHED:
        _CACHED[key] = _build_program(no_cc)
    return _CACHED[key]


def prep_in_maps(inputs):
    def f32(x):
        return np.ascontiguousarray(np.asarray(x, dtype=np.float32))

    def bf(x):
        return np.ascontiguousarray(np.asarray(x, dtype=np.float32).astype(ml_dtypes.bfloat16))

    source = np.asarray(inputs["source"]).astype(np.int32)
    emb = f32(inputs["emb"])
    ln_g, ln_b = f32(inputs["ln_g"]), f32(inputs["ln_b"])
    w1a, b1a = bf(inputs["w1"]), f32(inputs["b1"])
    w2a, b2a = bf(inputs["w2"]), f32(inputs["b2"])
    wqa, wka, wva = np.asarray(inputs["wq"]), np.asarray(inputs["wk"]), np.asarray(inputs["wv"])
    bqa, bka, bva = np.asarray(inputs["bq"]), np.asarray(inputs["bk"]), np.asarray(inputs["bv"])
    woa, boa = np.asarray(inputs["wo"]), f32(inputs["bo"])

    in_maps = []
    for core in range(8):
        b, half = core // 2, core % 2
        hsl = slice(half * DL, (half + 1) * DL)
        in_maps.append({
            "src": np.ascontiguousarray(source[b, half * TOWN:(half + 1) * TOWN]),
            "emb": emb,
            "wq": bf(wqa[:, :, hsl]), "wk": bf(wka[:, :, hsl]), "wv": bf(wva[:, :, hsl]),
            "bq": f32(bqa[:, hsl]), "bk": f32(bka[:, hsl]), "bv": f32(bva[:, hsl]),
            "wo": bf(woa[:, hsl, :]), "bo": boa,
            "w1": w1a, "b1": b1a, "w2": w2a, "b2": b2a,
            "ln_g": ln_g, "ln_b": ln_b,
        })
    return in_maps


def kernel(**inputs):
    nc = _get_program()
    in_maps = prep_in_maps(inputs)
    trace = bool(int(os.environ.get("BASS_ENC_TRACE", "0")))
    res = bass_utils.run_bass_kernel_spmd(nc, in_maps, core_ids=list(range(8)),
                                          trace=trace)
    _CACHED["last_results"] = res

    outp = np.empty((B, S, D), np.float32)
    for core in range(8):
        b, half = core // 2, core % 2
        outp[b, half * TOWN:(half + 1) * TOWN, :] = res.results[core]["out"]
    return outp


# revision 40
# speedup vs baseline: 1.0559x; 1.0167x over previous
"""Trainium2 Bass kernel for nn_Encoder (6-layer causal transformer encoder).

Sharding: 8 cores = 4 batch elements x 2-core tensor-parallel pairs.
Within a pair: attention is head-split (4 of 8 heads per core), FFN/LN/residual
are token-split (1024 of 2048 tokens per core).  Rank asymmetry is expressed
purely through ReduceScatter / AllGather rank order, so the SPMD program is
identical on every core.

v2 changes vs baseline:
 - collectives split into 2 token-chunks and reordered so they overlap
   compute (AG chunk issued right after its LN-B chunk; RS chunk issued
   after the two q-groups that feed it, overlapping the other two).
 - LayerNorm + softmax normalization broadcast via PE rank-1 outer products
   instead of DRAM round-trips.
 - w1/w2 SBUF-resident per layer (single load, no streaming stalls).
 - LN statistics matmuls run on float32r data at full PE rate (no bf16 copy).
 - score matmuls restricted to causally-valid columns in the diagonal band.
 - HW is PE-bound at the ~1.2GHz p-state, so the schedule keeps matmul
   streams dense and avoids GPSIMD software ops (slower on HW than modeled).
"""

import os
import sys

sys.path.insert(0, "/opt/trn_rl_repo")

import numpy as np
import ml_dtypes

import concourse.bass as bass
import concourse.mybir as mybir
import concourse.tile as tile
from concourse import bacc, bass_utils
from concourse.masks import make_identity, make_upper_triangular

# Problem constants (hardcoded per harness contract).
B, S, V, D, F, L = 4, 2048, 32000, 512, 2048, 6
H, Dh = 8, 64
HL = H // 2            # local heads per core (4)
DL = HL * Dh           # 256 local head-dims
TOWN = S // 2          # 1024 tokens owned per core
P = 128
CC = D // P            # 4 c-chunks
FC = F // P            # 16 f-chunks
CH = 512               # token chunk for collectives / LN / FFN
NCH = TOWN // CH       # 2 chunks
LN_EPS = 1e-5

FP32 = mybir.dt.float32
FP32R = mybir.dt.float32r
BF16 = mybir.dt.bfloat16
I32 = mybir.dt.int32

GROUPS = [[0, 1], [2, 3], [4, 5], [6, 7]]

_CACHED = {}


def _build_program(no_cc=False):
    nc = bacc.Bacc("TRN2", target_bir_lowering=False, debug=False, num_devices=8)
    if no_cc:
        # benchmarking variant: collectives replaced by a local DRAM copy
        # (wrong results; identical compute/DMA structure)
        def fake_cc(kind, op, replica_groups, ins, outs, **kw):
            src = ins[0]
            dst = outs[0]
            n = min(src.size(), dst.size())
            nc.sync.dma_start(
                out=bass.AP(tensor=dst.tensor, offset=dst.offset, ap=[[1, n]]),
                in_=bass.AP(tensor=src.tensor, offset=src.offset, ap=[[1, n]]))

        nc.gpsimd.collective_compute = fake_cc

    D_ = {}
    D_["src"] = nc.dram_tensor("src", [TOWN], I32, kind="ExternalInput")
    D_["emb"] = nc.dram_tensor("emb", [V, D], FP32, kind="ExternalInput")
    D_["wq"] = nc.dram_tensor("wq", [L, D, DL], BF16, kind="ExternalInput")
    D_["wk"] = nc.dram_tensor("wk", [L, D, DL], BF16, kind="ExternalInput")
    D_["wv"] = nc.dram_tensor("wv", [L, D, DL], BF16, kind="ExternalInput")
    D_["wo"] = nc.dram_tensor("wo", [L, DL, D], BF16, kind="ExternalInput")
    D_["bq"] = nc.dram_tensor("bq", [L, DL], FP32, kind="ExternalInput")
    D_["bk"] = nc.dram_tensor("bk", [L, DL], FP32, kind="ExternalInput")
    D_["bv"] = nc.dram_tensor("bv", [L, DL], FP32, kind="ExternalInput")
    D_["bo"] = nc.dram_tensor("bo", [L, D], FP32, kind="ExternalInput")
    D_["w1"] = nc.dram_tensor("w1", [L, D, F], BF16, kind="ExternalInput")
    D_["b1"] = nc.dram_tensor("b1", [L, F], FP32, kind="ExternalInput")
    D_["w2"] = nc.dram_tensor("w2", [L, F, D], BF16, kind="ExternalInput")
    D_["b2"] = nc.dram_tensor("b2", [L, D], FP32, kind="ExternalInput")
    D_["ln_g"] = nc.dram_tensor("ln_g", [D], FP32, kind="ExternalInput")
    D_["ln_b"] = nc.dram_tensor("ln_b", [D], FP32, kind="ExternalInput")
    D_["out"] = nc.dram_tensor("out", [TOWN, D], FP32, kind="ExternalOutput")

    # DRAM scratch, chunk-major so each collective chunk is contiguous.
    D_["xh"] = [nc.dram_tensor(f"xh{l}", [NCH, D, CH], BF16, kind="Internal")
                for l in range(L)]
    D_["xf"] = [nc.dram_tensor(f"xf{l}", [NCH, 2, D, CH], BF16, kind="Internal")
                for l in range(L)]
    D_["apart"] = [nc.dram_tensor(f"apart{l}", [NCH, 2, D, CH], BF16,
                                  kind="Internal")
                   for l in range(L)]
    D_["aown"] = [nc.dram_tensor(f"aown{l}", [NCH, D, CH], BF16, kind="Internal")
                  for l in range(L)]

    with tile.TileContext(nc) as tc:
        _emit(nc, tc, D_)

    nc.compile()
    return nc


def _emit(nc, tc, D_):
    from contextlib import ExitStack

    no_pool = bool(int(os.environ.get("BASS_ENC_NOPOOL", "1")))
    fine_attn = bool(int(os.environ.get("BASS_ENC_FINEATTN", "0")))
    ln_row = False  # row-broadcast LN variant deadlocked the tile scheduler

    ctx = ExitStack()
    Exp = mybir.ActivationFunctionType.Exp
    Relu = mybir.ActivationFunctionType.Relu
    Copy = mybir.ActivationFunctionType.Copy
    Ident = mybir.ActivationFunctionType.Identity
    Sqrt = mybir.ActivationFunctionType.Sqrt
    ADD = mybir.AluOpType.add
    SUB = mybir.AluOpType.subtract

    consts = ctx.enter_context(tc.tile_pool(name="consts", bufs=1))
    wpool = ctx.enter_context(tc.tile_pool(name="weights", bufs=1))
    stream = ctx.enter_context(tc.tile_pool(name="stream", bufs=1))
    acts = ctx.enter_context(tc.tile_pool(name="acts", bufs=1))
    xstrm = ctx.enter_context(tc.tile_pool(name="xstrm", bufs=2))
    chk = ctx.enter_context(tc.tile_pool(name="chunks", bufs=2))
    small = ctx.enter_context(tc.tile_pool(name="small", bufs=1))
    scratch = ctx.enter_context(tc.tile_pool(name="scratch", bufs=2))
    h1pool = ctx.enter_context(tc.tile_pool(name="h1pool", bufs=1))
    expp = ctx.enter_context(tc.tile_pool(name="exp", bufs=4))
    psA = ctx.enter_context(tc.tile_pool(
        name="psA",
        bufs=4 if bool(int(os.environ.get("BASS_ENC_FINEATTN", "0"))) else 2,
        space="PSUM"))
    psB = ctx.enter_context(tc.tile_pool(name="psB", bufs=2, space="PSUM"))
    psC = ctx.enter_context(tc.tile_pool(name="psC", bufs=2, space="PSUM"))

    # ---- constants ----
    ident = consts.tile([P, P], FP32)
    make_identity(nc, ident)
    trimask = consts.tile([P, P], BF16)  # 1 where k<=q
    make_upper_triangular(nc, trimask, val=1.0, diag=True)
    growf = consts.tile([1, D], FP32)
    nc.sync.dma_start(out=growf, in_=bass.AP(tensor=D_["ln_g"], offset=0,
                                             ap=[[0, 1], [1, D]]))
    grow = consts.tile([1, D], FP32R)    # ln_g as a row (outer-product lhsT)
    nc.vector.tensor_copy(out=grow, in_=growf)
    gTc = consts.tile([P, CC], FP32)     # ln_g column layout (per-partition)
    nc.sync.dma_start(out=gTc, in_=D_["ln_g"].ap().rearrange("(cc p) -> p cc", p=P))
    bT = consts.tile([P, CC], FP32)      # ln_b column layout (per-partition)
    nc.sync.dma_start(out=bT, in_=D_["ln_b"].ap().rearrange("(cc p) -> p cc", p=P))
    onesPf = consts.tile([P, 1], FP32)
    nc.vector.memset(onesPf, 1.0)
    onesP = consts.tile([P, 1], FP32R)
    nc.vector.tensor_copy(out=onesP, in_=onesPf)
    onesrowf = consts.tile([1, P], FP32)
    nc.vector.memset(onesrowf, 1.0)
    onesrow = consts.tile([1, P], FP32R)
    nc.vector.tensor_copy(out=onesrow, in_=onesrowf)
    epst = consts.tile([1, 1], FP32)
    nc.vector.memset(epst, LN_EPS)
    idx = consts.tile([P, TOWN // P], I32)
    nc.sync.dma_start(out=idx, in_=D_["src"].ap().rearrange("(tc p) -> p tc", p=P))

    onesPr, onesrowr, growr = onesP, onesrow, grow

    # ---- layer norm on one 512-token chunk, feature-major ----
    # s [P, CC, CH] fp32 -> writes normalized values into dest(cc) slices.
    # dest_fn(cc) returns the output AP for chunk cc.
    def layer_norm_chunk(s, dest_fn):
        mean = small.tile([1, CH], FP32, tag="lnmean", name="mean")
        msq = small.tile([1, CH], FP32, tag="lnmsq", name="msq")
        var = small.tile([1, CH], FP32, tag="lnvar", name="var")
        rstd = small.tile([1, CH], FP32R, tag="lnrstd", name="rstd")
        msrow = small.tile([1, CH], FP32R, tag="lnmsrow", name="msrow")
        ps_m = psC.tile([1, CH], FP32, tag="mm", name="ps_m")
        ps_q = psC.tile([1, CH], FP32, tag="mm", name="ps_q")
        for cc in range(CC):
            sq = scratch.tile([P, CH], FP32R, tag="scr", name="sq")
            eng = nc.vector if no_pool else nc.gpsimd
            eng.tensor_mul(out=sq, in0=s[:, cc, :], in1=s[:, cc, :])
            nc.tensor.matmul(ps_m, onesPr, s[:, cc, :],
                             start=(cc == 0), stop=(cc == CC - 1))
            nc.tensor.matmul(ps_q, onesPr, sq,
                             start=(cc == 0), stop=(cc == CC - 1))
        nc.scalar.activation(out=mean, in_=ps_m, func=Copy, scale=1.0 / D)
        nc.scalar.activation(out=msq, in_=ps_q, func=Copy, scale=1.0 / D)
        nc.vector.tensor_mul(out=var, in0=mean, in1=mean)
        nc.vector.tensor_sub(out=var, in0=msq, in1=var)
        nc.scalar.activation(out=rstd, in_=var, func=Sqrt, bias=epst, scale=1.0)
        with nc.allow_low_precision(reason="fp32r rstd feeds fp32r matmul broadcast"):
            nc.vector.reciprocal(out=rstd, in_=rstd)
        nc.vector.tensor_mul(out=msrow, in0=mean, in1=rstd)
        if ln_row:
            # 2 row-broadcasts (copied straight to SBUF to keep psum free),
            # then classic 3-op normalize per cc: moves work from the
            # saturated PE to the underused DVE.
            mBp = psC.tile([P, CH], FP32, tag="mm", name="mBp")
            nc.tensor.matmul(mBp, onesrow, mean, start=True, stop=True)
            mBs = scratch.tile([P, CH], FP32, tag="scr", name="mBs")
            nc.vector.tensor_copy(out=mBs, in_=mBp)
            rBp = psC.tile([P, CH], FP32, tag="mm", name="rBp")
            nc.tensor.matmul(rBp, onesrow, rstd, start=True, stop=True)
            rBs = scratch.tile([P, CH], FP32, tag="scr", name="rBs")
            nc.vector.tensor_copy(out=rBs, in_=rBp)
            for cc in range(CC):
                t = scratch.tile([P, CH], FP32, tag="scr", name="t")
                nc.vector.tensor_sub(out=t, in0=s[:, cc, :], in1=mBs)
                nc.vector.tensor_mul(out=t, in0=t, in1=rBs)
                nc.vector.tensor_scalar(out=dest_fn(cc), in0=t,
                                        scalar1=gTc[:, cc:cc + 1],
                                        scalar2=bT[:, cc:cc + 1],
                                        op0=mybir.AluOpType.mult, op1=ADD)
        else:
            for cc in range(CC):
                gsl = growr[0:1, cc * P:(cc + 1) * P]
                sgB = psC.tile([P, CH], FP32, tag="mm", name="sgB")
                nc.tensor.matmul(sgB, gsl, rstd, start=True, stop=True)
                gmsB = psC.tile([P, CH], FP32, tag="mm", name="gmsB")
                nc.tensor.matmul(gmsB, gsl, msrow, start=True, stop=True)
                t = scratch.tile([P, CH], FP32, tag="scr", name="t")
                nc.vector.tensor_mul(out=t, in0=s[:, cc, :], in1=sgB)
                # dest = (t + b) - g*mean*rstd
                nc.vector.scalar_tensor_tensor(
                    out=dest_fn(cc), in0=t, scalar=bT[:, cc:cc + 1], in1=gmsB,
                    op0=ADD, op1=SUB)

    # ---- embedding gather for own tokens -> x_own [P, CC, TOWN] fp32 ----
    x_own = stream.tile([P, CC, TOWN], FP32, tag="x_own")
    for c in range(NCH):
        xhbc = chk.tile([P, CC, CH], BF16, tag="xhbc", name="xhbc")
        for tb in range(CH // P):
            tcN = c * (CH // P) + tb
            rows = xstrm.tile([P, D], FP32, tag="erows", name="erows")
            nc.gpsimd.indirect_dma_start(
                out=rows, out_offset=None, in_=D_["emb"].ap(),
                in_offset=bass.IndirectOffsetOnAxis(ap=idx[:, tcN:tcN + 1], axis=0))
            for cc in range(CC):
                pt = psC.tile([P, P], FP32, tag="mm", name="pt")
                nc.tensor.transpose(pt, rows[:, cc * P:(cc + 1) * P], ident)
                sl = slice(tcN * P, (tcN + 1) * P)
                nc.vector.tensor_copy(out=x_own[:, cc, sl], in_=pt)
                eng = nc.vector if no_pool else nc.gpsimd
                eng.tensor_copy(out=xhbc[:, cc, tb * P:(tb + 1) * P],
                                in_=x_own[:, cc, sl])
        nc.sync.dma_start(
            out=D_["xh"][0].ap()[c].rearrange("(cc p) t -> p cc t", p=P),
            in_=xhbc)
        nc.gpsimd.collective_compute(
            kind="AllGather", op=mybir.AluOpType.bypass, replica_groups=GROUPS,
            ins=[D_["xh"][0].ap()[c]], outs=[D_["xf"][0].ap()[c]])

    # ---- per-layer weight loads ----
    def emit_weights_qkv(l):
        W = {}
        W["wq"] = wpool.tile([P, CC, DL], BF16, tag="wq", name="wq_t")
        nc.sync.dma_start(out=W["wq"], in_=D_["wq"].ap()[l].rearrange("(cc p) d -> p cc d", p=P))
        W["wk"] = wpool.tile([P, CC, DL], BF16, tag="wk", name="wk_t")
        nc.sync.dma_start(out=W["wk"], in_=D_["wk"].ap()[l].rearrange("(cc p) d -> p cc d", p=P))
        W["wv"] = wpool.tile([P, CC, DL], BF16, tag="wv", name="wv_t")
        nc.sync.dma_start(out=W["wv"], in_=D_["wv"].ap()[l].rearrange("(cc p) d -> p cc d", p=P))
        W["wo"] = wpool.tile([P, 2, D], BF16, tag="wo", name="wo_t")
        nc.sync.dma_start(out=W["wo"], in_=D_["wo"].ap()[l].rearrange("(hc p) d -> p hc d", p=P))
        W["bq"] = wpool.tile([P, 2], FP32, tag="bq", name="bq_t")
        nc.sync.dma_start(out=W["bq"], in_=D_["bq"].ap()[l].rearrange("(hc p) -> p hc", p=P))
        W["bk"] = wpool.tile([P, 2], FP32, tag="bk", name="bk_t")
        nc.sync.dma_start(out=W["bk"], in_=D_["bk"].ap()[l].rearrange("(hc p) -> p hc", p=P))
        W["bv"] = wpool.tile([P, DL], FP32, tag="bvB", name="bvB")
        nc.sync.dma_start(out=W["bv"], in_=bass.AP(tensor=D_["bv"], offset=l * DL,
                                                   ap=[[0, P], [1, DL]]))
        W["bo"] = wpool.tile([P, CC], FP32, tag="bo", name="bo_t")
        nc.sync.dma_start(out=W["bo"], in_=D_["bo"].ap()[l].rearrange("(cc p) -> p cc", p=P))
        return W

    def emit_weights_ffn(l, W):
        W["w1"] = wpool.tile([P, CC, F], BF16, tag="w1S", name="w1S")
        nc.sync.dma_start(out=W["w1"], in_=D_["w1"].ap()[l].rearrange("(cc p) f -> p cc f", p=P))
        W["w2"] = wpool.tile([P, FC, D], BF16, tag="w2S", name="w2S")
        nc.sync.dma_start(out=W["w2"], in_=D_["w2"].ap()[l].rearrange("(fc p) d -> p fc d", p=P))
        W["b1"] = wpool.tile([P, FC], FP32, tag="b1", name="b1_t")
        nc.sync.dma_start(out=W["b1"], in_=D_["b1"].ap()[l].rearrange("(fc p) -> p fc", p=P))
        W["b2"] = wpool.tile([P, CC], FP32, tag="b2", name="b2_t")
        nc.sync.dma_start(out=W["b2"], in_=D_["b2"].ap()[l].rearrange("(cc p) -> p cc", p=P))

    def qkv_tg(l, tg, W, T):
        if "QT" not in T:
            T["QT"] = acts.tile([P, 2, S], BF16, tag="QT", name="QT")
            T["KT"] = acts.tile([P, 2, S], BF16, tag="KT", name="KT")
            T["VR"] = acts.tile([P, S // P, HL, Dh + 1], BF16, tag="VR", name="VR")
            nc.vector.memset(T["VR"][:, :, :, Dh:Dh + 1], 1.0)
        c, r = tg % 2, tg // 2
        xbt = xstrm.tile([P, CC, CH], BF16, tag="xbt", name="xbt")
        nc.sync.dma_start(
            out=xbt,
            in_=D_["xf"][l].ap()[c, r].rearrange("(cc p) t -> p cc t", p=P))
        sl = slice(tg * CH, (tg + 1) * CH)
        for dst, w_t, b_t in ((T["QT"], W["wq"], W["bq"]), (T["KT"], W["wk"], W["bk"])):
            for hc in range(2):
                ps = psC.tile([P, CH], FP32, tag="mm", name="ps_qk")
                for cc in range(CC):
                    nc.tensor.matmul(
                        ps, w_t[:, cc, hc * P:(hc + 1) * P], xbt[:, cc, :],
                        start=(cc == 0), stop=(cc == CC - 1))
                nc.scalar.activation(out=dst[:, hc, sl], in_=ps, func=Ident,
                                     bias=b_t[:, hc:hc + 1], scale=1.0)
        for tb in range(CH // P):
            tcN = tg * (CH // P) + tb
            ps = psC.tile([P, DL], FP32, tag="mm", name="ps_v")
            for cc in range(CC):
                nc.tensor.matmul(
                    ps, xbt[:, cc, tb * P:(tb + 1) * P], W["wv"][:, cc, :],
                    start=(cc == 0), stop=(cc == CC - 1))
            nc.vector.tensor_add(
                out=T["VR"][:, tcN, :, 0:Dh],
                in0=ps.rearrange("p (h d) -> p h d", h=HL),
                in1=W["bv"].rearrange("p (h d) -> p h d", h=HL))

    def attn_head(qg, h, T):
        if "attnT" not in T:
            T["attnT"] = acts.tile([P, 2, S], BF16, tag="attnT", name="attnT")
        qsl = slice(qg * CH, (qg + 1) * CH)
        kmax = qg * 4 + 3
        hp, ho = h // 2, (h % 2) * Dh
        qt_h = T["QT"][ho:ho + Dh, hp, :]
        kt_h = T["KT"][ho:ho + Dh, hp, :]
        av = psB.tile([Dh + 1, CH], FP32, tag="av", name="av")
        if fine_attn:
            for kb in range(kmax + 1):
                koff = max(0, kb - qg * 4) * P
                sc = psA.tile([P, CH], FP32, tag="sc", name="sc")
                nc.tensor.matmul(sc[:, koff:],
                                 kt_h[:, kb * P:(kb + 1) * P],
                                 qt_h[:, qg * CH + koff:(qg + 1) * CH],
                                 start=True, stop=True)
                ex = expp.tile([P, CH], BF16, tag="ex", name="ex")
                nc.scalar.activation(out=ex[:, koff:], in_=sc[:, koff:],
                                     func=Exp, scale=1.0 / 8.0)
                dj = kb - qg * 4
                if 0 <= dj <= 3:
                    nc.vector.tensor_mul(out=ex[:, dj * P:(dj + 1) * P],
                                         in0=ex[:, dj * P:(dj + 1) * P],
                                         in1=trimask)
                nc.tensor.matmul(av[:, koff:], T["VR"][:, kb, h, :], ex[:, koff:],
                                 start=(kb == 0), stop=(kb == kmax))
        else:
            for kb0 in range(0, kmax + 1, 2):
                sc = psA.tile([P, 1024], FP32, tag="sc", name="sc")
                for j in range(2):
                    koff = max(0, kb0 + j - qg * 4) * P
                    nc.tensor.matmul(sc[:, j * CH + koff:(j + 1) * CH],
                                     kt_h[:, (kb0 + j) * P:(kb0 + j + 1) * P],
                                     qt_h[:, qg * CH + koff:(qg + 1) * CH],
                                     start=True, stop=True)
                ex = expp.tile([P, 1024], BF16, tag="ex", name="ex")
                off0 = max(0, kb0 - qg * 4) * P
                nc.scalar.activation(out=ex[:, off0:], in_=sc[:, off0:],
                                     func=Exp, scale=1.0 / 8.0)
                for j in range(2):
                    kb = kb0 + j
                    exj = ex[:, j * CH:(j + 1) * CH]
                    dj = kb - qg * 4
                    if 0 <= dj <= 3:  # diagonal block: apply causal mask
                        nc.vector.tensor_mul(out=exj[:, dj * P:(dj + 1) * P],
                                             in0=exj[:, dj * P:(dj + 1) * P],
                                             in1=trimask)
                    off = max(0, dj) * P
                    nc.tensor.matmul(av[:, off:], T["VR"][:, kb, h, :], exj[:, off:],
                                     start=(kb == 0), stop=(kb == kmax))
        # normalize: attnT = av[0:Dh] * broadcast(1/av[Dh])
        if no_pool:
            rrow = small.tile([1, CH], FP32R, tag="rrow", name="rrow")
            with nc.allow_low_precision(reason="softmax recip row"):
                nc.vector.reciprocal(out=rrow, in_=av[Dh:Dh + 1, :])
            rbt = psC.tile([Dh, CH], FP32, tag="mm", name="rbt")
            nc.tensor.matmul(rbt, onesrow[0:1, 0:Dh], rrow, start=True, stop=True)
            att_sl = T["attnT"][ho:ho + Dh, hp, qsl]
            nc.vector.tensor_copy(out=att_sl, in_=av[0:Dh, :])
            nc.vector.tensor_mul(out=att_sl, in0=att_sl, in1=rbt)
        else:
            rrow = small.tile([1, CH], FP32, tag="rrow", name="rrow")
            nc.vector.reciprocal(out=rrow, in_=av[Dh:Dh + 1, :])
            rbt = scratch.tile([Dh, CH], FP32, tag="rbt", name="rbt")
            nc.gpsimd.partition_broadcast(out_ap=rbt, in_ap=rrow)
            nc.vector.tensor_mul(out=T["attnT"][ho:ho + Dh, hp, qsl],
                                 in0=av[0:Dh, :], in1=rbt)

    def opart(l, qg, W, T):
        qsl = slice(qg * CH, (qg + 1) * CH)
        c, half = qg % 2, qg // 2
        for dc in range(CC):
            ps = psC.tile([P, CH], FP32, tag="mm", name="ps_o")
            for hc in range(2):
                nc.tensor.matmul(ps, W["wo"][:, hc, dc * P:(dc + 1) * P],
                                 T["attnT"][:, hc, qsl],
                                 start=(hc == 0), stop=(hc == 1))
            ob = scratch.tile([P, CH], BF16, tag="ob", name="ob")
            nc.vector.tensor_copy(out=ob, in_=ps)
            nc.sync.dma_start(
                out=D_["apart"][l].ap()[c, half, dc * P:(dc + 1) * P, :],
                in_=ob)

    def rs(l, c):
        nc.gpsimd.collective_compute(
            kind="ReduceScatter", op=ADD, replica_groups=GROUPS,
            ins=[D_["apart"][l].ap()[c]], outs=[D_["aown"][l].ap()[c]])

    def resid1_lna(l, c, W, T):
        if "yb" not in T:
            T["yb"] = stream.tile([P, CC, TOWN], BF16, tag="yb", name="yb")
        csl = slice(c * CH, (c + 1) * CH)
        ar = chk.tile([P, CC, CH], BF16, tag="ar", name="ar")
        nc.sync.dma_start(
            out=ar, in_=D_["aown"][l].ap()[c].rearrange("(cc p) t -> p cc t", p=P))
        s1 = chk.tile([P, CC, CH], FP32R, tag="sres", name="s1")
        for cc in range(CC):
            nc.vector.scalar_tensor_tensor(
                out=s1[:, cc, :], in0=ar[:, cc, :], scalar=W["bo"][:, cc:cc + 1],
                in1=x_own[:, cc, csl], op0=ADD, op1=ADD)
        yb = T["yb"]
        layer_norm_chunk(s1, lambda cc, _csl=csl: yb[:, cc, _csl])

    def ffn(l, c, W, T):
        csl = slice(c * CH, (c + 1) * CH)
        h1T = h1pool.tile([P, FC, CH], BF16, tag="h1T", name="h1T")
        for fc in range(FC):
            ps = psC.tile([P, CH], FP32, tag="mm", name="ps_f1")
            for cc in range(CC):
                nc.tensor.matmul(ps, W["w1"][:, cc, fc * P:(fc + 1) * P],
                                 T["yb"][:, cc, csl],
                                 start=(cc == 0), stop=(cc == CC - 1))
            nc.scalar.activation(out=h1T[:, fc, :], in_=ps, func=Relu,
                                 bias=W["b1"][:, fc:fc + 1])
        fob = chk.tile([P, CC, CH], BF16, tag="fob", name="fob")
        for dc in range(CC):
            ps = psC.tile([P, CH], FP32, tag="mm", name="ps_f2")
            for fc in range(FC):
                nc.tensor.matmul(ps, W["w2"][:, fc, dc * P:(dc + 1) * P],
                                 h1T[:, fc, :],
                                 start=(fc == 0), stop=(fc == FC - 1))
            nc.scalar.activation(out=fob[:, dc, :], in_=ps, func=Relu,
                                 bias=W["b2"][:, dc:dc + 1])
        return fob

    def resid2_lnb_ag(l, c, fob, T):
        csl = slice(c * CH, (c + 1) * CH)
        s2 = chk.tile([P, CC, CH], FP32R, tag="sres", name="s2")
        for cc in range(CC):
            nc.vector.tensor_add(out=s2[:, cc, :], in0=T["yb"][:, cc, csl],
                                 in1=fob[:, cc, :])
        layer_norm_chunk(s2, lambda cc, _csl=csl: x_own[:, cc, _csl])
        if l < L - 1:
            xhbc = chk.tile([P, CC, CH], BF16, tag="xhbc", name="xhbc")
            eng = nc.vector if no_pool else nc.gpsimd
            for cc in range(CC):
                eng.tensor_copy(out=xhbc[:, cc, :], in_=x_own[:, cc, csl])
            nc.sync.dma_start(
                out=D_["xh"][l + 1].ap()[c].rearrange("(cc p) t -> p cc t", p=P),
                in_=xhbc)
            nc.gpsimd.collective_compute(
                kind="AllGather", op=mybir.AluOpType.bypass,
                replica_groups=GROUPS,
                ins=[D_["xh"][l + 1].ap()[c]], outs=[D_["xf"][l + 1].ap()[c]])

    def out_rows(tb):
        rows = xstrm.tile([P, D], FP32, tag="orows", name="orows")
        for cc in range(CC):
            pt = psC.tile([P, P], FP32, tag="mm", name="pt_o")
            nc.tensor.transpose(pt, x_own[:, cc, tb * P:(tb + 1) * P], ident)
            nc.vector.tensor_copy(out=rows[:, cc * P:(cc + 1) * P], in_=pt)
        nc.sync.dma_start(out=D_["out"].ap()[tb * P:(tb + 1) * P, :], in_=rows)

    # ---- layer schedule (two variants, BASS_ENC_SCHED picks) ----
    sched = os.environ.get("BASS_ENC_SCHED", "v1")
    if sched == "v1":
        for l in range(L):
            W = emit_weights_qkv(l)
            T = {}
            qkv_tg(l, 0, W, T)
            qkv_tg(l, 2, W, T)
            for h in range(HL):
                attn_head(0, h, T)
            opart(l, 0, W, T)
            qkv_tg(l, 1, W, T)
            qkv_tg(l, 3, W, T)
            for h in range(HL):
                attn_head(2, h, T)
            opart(l, 2, W, T)
            rs(l, 0)
            emit_weights_ffn(l, W)
            for h in range(HL):
                attn_head(1, h, T)
            opart(l, 1, W, T)
            for h in range(HL):
                attn_head(3, h, T)
            opart(l, 3, W, T)
            rs(l, 1)
            resid1_lna(l, 0, W, T)
            fob0 = ffn(l, 0, W, T)
            resid2_lnb_ag(l, 0, fob0, T)
            if l == L - 1:
                # chunk-0 output transposes fill the last RS1 wait window
                for tb in range(TOWN // (2 * P)):
                    out_rows(tb)
            resid1_lna(l, 1, W, T)
            fob1 = ffn(l, 1, W, T)
            resid2_lnb_ag(l, 1, fob1, T)
        for tb in range(TOWN // (2 * P), TOWN // P):
            out_rows(tb)
    else:
        W = emit_weights_qkv(0)
        T = {}
        qkv_tg(0, 0, W, T)
        qkv_tg(0, 2, W, T)
        for h in range(HL):
            attn_head(0, h, T)
        opart(0, 0, W, T)
        for l in range(L):
            last = l == L - 1
            emit_weights_ffn(l, W)
            qkv_tg(l, 1, W, T)
            qkv_tg(l, 3, W, T)
            for h in range(HL):
                attn_head(2, h, T)
            opart(l, 2, W, T)
            rs(l, 0)
            for h in range(HL):
                attn_head(3, h, T)
            opart(l, 3, W, T)
            resid1_lna(l, 0, W, T)
            fob0 = ffn(l, 0, W, T)
            resid2_lnb_ag(l, 0, fob0, T)
            for h in range(HL):
                attn_head(1, h, T)
            opart(l, 1, W, T)
            rs(l, 1)
            if not last:
                Wn = emit_weights_qkv(l + 1)
                Tn = {}
                qkv_tg(l + 1, 0, Wn, Tn)
                qkv_tg(l + 1, 2, Wn, Tn)
                attn_head(0, 0, Tn)
                attn_head(0, 1, Tn)
            else:
                for tb in range(TOWN // (2 * P)):
                    out_rows(tb)
            resid1_lna(l, 1, W, T)
            fob1 = ffn(l, 1, W, T)
            resid2_lnb_ag(l, 1, fob1, T)
            if not last:
                attn_head(0, 2, Tn)
                attn_head(0, 3, Tn)
                opart(l + 1, 0, Wn, Tn)
                W, T = Wn, Tn
        for tb in range(TOWN // (2 * P), TOWN // P):
            out_rows(tb)

    ctx.close()


def _get_program():
    no_cc = bool(int(os.environ.get("BASS_ENC_NOCC", "0")))
    key = ("nc", no_cc)
    if key not in _CACHED:
        _CACHED[key] = _build_program(no_cc)
    return _CACHED[key]


def prep_in_maps(inputs):
    def f32(x):
        return np.ascontiguousarray(np.asarray(x, dtype=np.float32))

    def bf(x):
        return np.ascontiguousarray(np.asarray(x, dtype=np.float32).astype(ml_dtypes.bfloat16))

    source = np.asarray(inputs["source"]).astype(np.int32)
    emb = f32(inputs["emb"])
    ln_g, ln_b = f32(inputs["ln_g"]), f32(inputs["ln_b"])
    w1a, b1a = bf(inputs["w1"]), f32(inputs["b1"])
    w2a, b2a = bf(inputs["w2"]), f32(inputs["b2"])
    wqa, wka, wva = np.asarray(inputs["wq"]), np.asarray(inputs["wk"]), np.asarray(inputs["wv"])
    bqa, bka, bva = np.asarray(inputs["bq"]), np.asarray(inputs["bk"]), np.asarray(inputs["bv"])
    woa, boa = np.asarray(inputs["wo"]), f32(inputs["bo"])

    in_maps = []
    for core in range(8):
        b, half = core // 2, core % 2
        hsl = slice(half * DL, (half + 1) * DL)
        in_maps.append({
            "src": np.ascontiguousarray(source[b, half * TOWN:(half + 1) * TOWN]),
            "emb": emb,
            "wq": bf(wqa[:, :, hsl]), "wk": bf(wka[:, :, hsl]), "wv": bf(wva[:, :, hsl]),
            "bq": f32(bqa[:, hsl]), "bk": f32(bka[:, hsl]), "bv": f32(bva[:, hsl]),
            "wo": bf(woa[:, hsl, :]), "bo": boa,
            "w1": w1a, "b1": b1a, "w2": w2a, "b2": b2a,
            "ln_g": ln_g, "ln_b": ln_b,
        })
    return in_maps


def kernel(**inputs):
    nc = _get_program()
    in_maps = prep_in_maps(inputs)
    trace = bool(int(os.environ.get("BASS_ENC_TRACE", "0")))
    res = bass_utils.run_bass_kernel_spmd(nc, in_maps, core_ids=list(range(8)),
                                          trace=trace)
    _CACHED["last_results"] = res

    outp = np.empty((B, S, D), np.float32)
    for core in range(8):
        b, half = core // 2, core % 2
        outp[b, half * TOWN:(half + 1) * TOWN, :] = res.results[core]["out"]
    return outp
